# revision 9
# baseline (speedup 1.0000x reference)
"""Trainium2 8-core tensor-parallel Llama3-style GQA attention layer.

Problem: B=1, S=2048, D=4096, H=32 Q heads, KVH=8 KV heads, HD=128,
interleaved-pair RoPE (theta=5e5), causal softmax, output projection.

Sharding (Megatron TP-8):
  - core c owns Q heads [4c..4c+3] and KV head c (GQA groups align exactly),
  - x is replicated (passed pre-transposed as xT so the d-contraction sits on
    partitions with no on-device transposes),
  - wq/wk rows are permuted per head (even pair-indices first, then odd) so the
    interleaved RoPE becomes a "rotate-half" that is partition-aligned; the
    permutation cancels inside the q.k dot product,
  - weights are staged in DRAM partition-major ([128, ko, m]) so weight DMAs
    read 2-8KB contiguous lines instead of 256B gathers.

Schedule: projections (A_j) and attention (B_j) are interleaved per s-chunk
so each chunk's attention output stages ~130us earlier and the serialized
AllGather stream (~45-55us per op) fully hides under compute; the gathered
attention is consumed by a column-parallel wo projection (phase C) whose
input loads ride the same SBUF slots as the x chunks.

Attention details: scoresT[s2,s1] layout; the two heads of a GQA pair share
one [128, 2*CHUNK] score PSUM tile so exp is one ACT call per block; the
causal mask multiply only touches the 128 diagonal columns; softmax
denominators accumulate in bf16 on the DVE (off the TensorEngine) and are
partition-reduced by one tiny ones-matmul per (head, chunk); PSUM evacuation
runs on the scalar engine.  All of B's non-matmul work overlaps the next
chunk's projection matmuls.

kernel(**inputs) takes the FULL fp32 inputs and returns the FULL fp32 output.
"""

import sys

sys.path.insert(0, "/opt/trn_rl_repo")

import math

import numpy as np
import ml_dtypes

import concourse.bass as bass  # noqa: F401
import concourse.mybir as mybir
import concourse.tile as tile
from concourse import bacc
from concourse.bass_utils import run_bass_kernel_spmd
from concourse.masks import make_identity

bf16 = ml_dtypes.bfloat16
F32 = mybir.dt.float32
BF16 = mybir.dt.bfloat16

# Problem shapes (hardcoded per spec)
B, S, D = 1, 2048, 4096
H, KVH, HD = 32, 8, 128
NCORES = 8
HLOC = H // NCORES            # 4 q heads per core
ELOC = HLOC * HD              # 512 attn-out dims per core
NKO = D // 128                # 32 k-tiles of the d contraction
CHUNK = 512                   # s-chunk (matmul free dim / psum bank)
NCHUNK = S // CHUNK           # 4
NB = S // 128                 # 16 s2 blocks
SCALE = 1.0 / math.sqrt(HD)
NWARM = 44                    # HAM-prewarm matmuls issued during startup DMAs

_NC_CACHE = None


def _build():
    nc = bacc.Bacc(
        "TRN2",
        target_bir_lowering=False,
        debug=False,
        enable_asserts=True,
        num_devices=NCORES,
    )
    xT_e = nc.dram_tensor("xT", [D, S], BF16, kind="ExternalInput")
    # weights staged partition-major: [p, ko, m] so DMA lines are contiguous
    wq_e = nc.dram_tensor("wqT", [128, NKO, ELOC], BF16, kind="ExternalInput")
    wk_e = nc.dram_tensor("wkT", [128, NKO, HD], BF16, kind="ExternalInput")
    wv_e = nc.dram_tensor("wvT", [128, NKO, HD], BF16, kind="ExternalInput")
    wo_e = nc.dram_tensor("woT", [128, NKO, ELOC], BF16, kind="ExternalInput")
    cos_e = nc.dram_tensor("cosT", [HD, S], F32, kind="ExternalInput")
    sin_e = nc.dram_tensor("sinT", [HD, S], F32, kind="ExternalInput")
    out_e = nc.dram_tensor("out", [ELOC, S], F32, kind="ExternalOutput")

    xT = xT_e.ap().rearrange("(ko p) s -> p ko s", p=128)       # [128, 32, 2048]
    wqT = wq_e.ap()                                             # [128, 32, 512]
    wkT = wk_e.ap()                                             # [128, 32, 128]
    wvT = wv_e.ap()
    woT = wo_e.ap()                                             # [128, 32, 512]

    rg = [list(range(NCORES))]

    with tile.TileContext(nc) as tc:
        with (
            tc.tile_pool(name="dram", bufs=1, space="DRAM") as dram_pool,
            tc.tile_pool(name="persist", bufs=1) as pp,
        ):
            # AllGather buffers: one 2-chunk AG for s-chunks 0+1, then one
            # per chunk for 2 and 3; each fires as soon as its staging lands.
            AG_WIDTHS = [2 * CHUNK, CHUNK, CHUNK]
            ag_in = [
                dram_pool.tile([ELOC, w], BF16, name=f"ag_in{k}")
                for k, w in enumerate(AG_WIDTHS)
            ]
            ag_out = [
                dram_pool.tile(
                    [NCORES * ELOC, w], BF16, name=f"ag_out{k}",
                    addr_space="Shared",
                )
                for k, w in enumerate(AG_WIDTHS)
            ]

            def ag_slot(j):
                """(ag index, column offset) for s-chunk j."""
                return (0, j * CHUNK) if j < 2 else (j - 1, 0)

            # ---- small constants ----
            ident = pp.tile([128, 128], BF16)
            make_identity(nc, ident[:])
            band = pp.tile([128, 896], BF16)
            nc.gpsimd.memset(band[:], 1.0)
            # band[p, u] = 1 iff u >= p + 384
            nc.gpsimd.affine_select(
                out=band[:], in_=band[:],
                compare_op=mybir.AluOpType.is_ge, fill=0.0,
                base=-384, channel_multiplier=-1, pattern=[[1, 896]],
            )
            ones_sb = pp.tile([128, 1], BF16)
            nc.gpsimd.memset(ones_sb[:], 1.0)

            cos_sb = pp.tile([128, S], F32)
            sin_sb = pp.tile([128, S], F32)

            # ---- persistent activations ----
            qsb = pp.tile([128, HLOC, S], BF16)     # roped qT per head
            ksb = pp.tile([128, S], BF16)           # roped kT
            vsb = pp.tile([128, NB, HD], BF16)      # v[s2-tile, :, hd]

            ag_tiles = {}
            staging_last = {}

            # x-chunk pool outlives the A/B pools: the phase-C gathered-attn
            # loads ride the same tags/slots (so their SBUF region is never
            # reused under them by the later wo pool).
            with tc.tile_pool(name="xch", bufs=1) as xp:

                last_xc_dma = {}

                def load_xchunk(j, fine=False):
                    js = slice(j * CHUNK, (j + 1) * CHUNK)
                    xc_g = []
                    for g in range(4):
                        t = xp.tile(
                            [128, 8, CHUNK], BF16, tag=f"xc{g}", bufs=2,
                            name=f"xc{j}_{g}",
                        )
                        eng = nc.gpsimd if g % 2 else nc.sync
                        step = 1 if (fine and g == 0) else (2 if fine else 4)
                        for s0 in range(0, 8, step):
                            dd = eng.dma_start(
                                t[:, s0:s0 + step, :],
                                xT[:, 8 * g + s0:8 * g + s0 + step, js],
                            )
                            last_xc_dma[(j, "gps" if g % 2 else "sync")] = dd
                        xc_g.append(t)
                    return xc_g

                def load_ag(cj, eng, gate):
                    """Gathered-attn chunk cj -> SBUF, riding the xc tags.
                    `gate` orders these after critical work in eng's FIFO
                    (they block on AllGather completion at the queue head)."""
                    k, co = ag_slot(cj)
                    agt = ag_out[k][:].rearrange("(ko p) s -> p ko s", p=128)
                    tiles = []
                    for g in range(4):
                        t = xp.tile(
                            [128, 8, CHUNK], BF16, tag=f"xc{g}", bufs=2,
                            name=f"agsb{cj}_{g}",
                        )
                        d1 = eng.dma_start(
                            t[:, 0:4, :], agt[:, 8 * g:8 * g + 4, co:co + CHUNK]
                        )
                        d2 = eng.dma_start(
                            t[:, 4:8, :],
                            agt[:, 8 * g + 4:8 * g + 8, co:co + CHUNK],
                        )
                        if gate is not None:
                            for dd in (d1, d2):
                                tile.add_dep_helper(
                                    dd.ins, gate.ins, sync=False,
                                    reason="ag loads after critical queue work",
                                )
                        tiles.append(t)
                    return tiles

                with (
                    tc.tile_pool(name="wq", bufs=1) as wqp,
                    tc.tile_pool(name="wkv", bufs=1) as wkvp,
                    tc.tile_pool(name="rope", bufs=2) as rp,
                    tc.tile_pool(name="pt", bufs=6) as ptp,
                    tc.tile_pool(name="misc", bufs=2) as mp,
                    tc.tile_pool(name="stage", bufs=3) as stp,
                    tc.tile_pool(name="ps", bufs=1, space="PSUM") as ps,
                ):
                    def grp_load(pool, dram_t, m, name, nsplit=1,
                                 gate=None):
                        tiles = []
                        for g in range(4):
                            t = pool.tile([128, 8, m], BF16, name=f"{name}{g}")
                            step = 8 // nsplit
                            for s0 in range(0, 8, step):
                                dd = nc.sync.dma_start(
                                    t[:, s0:s0 + step, :],
                                    dram_t[:, 8 * g + s0:8 * g + s0 + step, :],
                                )
                                if gate is not None:
                                    tile.add_dep_helper(
                                        dd.ins, gate.ins, sync=False,
                                        reason="weights after x chunk 0",
                                    )
                            tiles.append(t)
                        return tiles

                    # DMA issue order = queue service order: wk first (first
                    # consumer), then x chunk 0, then wq/cos/sin/wv (gated
                    # after xc0 so the scheduler can't starve the k chain).
                    wk_g = grp_load(wkvp, wkT, HD, "wk", nsplit=1)
                    xc0_g = load_xchunk(0, fine=True)
                    xc0_gate = last_xc_dma[(0, "sync")]
                    wq_g = grp_load(wqp, wqT, ELOC, "wq", nsplit=2,
                                    gate=xc0_gate)
                    for g in range(4):
                        sl = slice(g * 512, (g + 1) * 512)
                        for src_ap, dst in ((cos_e, cos_sb), (sin_e, sin_sb)):
                            dd = nc.sync.dma_start(
                                dst[:, sl], src_ap.ap()[:, sl]
                            )
                            tile.add_dep_helper(
                                dd.ins, xc0_gate.ins, sync=False,
                                reason="cos/sin after x chunk 0",
                            )
                    wv_g = grp_load(wkvp, wvT, HD, "wv", nsplit=1,
                                    gate=xc0_gate)

                    def rope(dst01, src_ps, js):
                        """dst01: (ap_lo, ap_hi) bf16 targets [64, 512] each.
                        src_ps: [128, 512] psum with the permuted projection."""
                        tc_t = rp.tile([128, CHUNK], F32, tag="ropec")
                        ts_t = rp.tile([128, CHUNK], F32, tag="ropes")
                        sw_t = rp.tile([128, CHUNK], F32, tag="ropew")
                        nc.vector.tensor_mul(tc_t[:], src_ps[:], cos_sb[:, js])
                        nc.vector.tensor_mul(ts_t[:], src_ps[:], sin_sb[:, js])
                        nc.gpsimd.dma_start(sw_t[0:64, :], ts_t[64:128, :])
                        nc.gpsimd.dma_start(sw_t[64:128, :], ts_t[0:64, :])
                        nc.vector.tensor_sub(dst01[0], tc_t[0:64, :], sw_t[0:64, :])
                        nc.vector.tensor_add(dst01[1], tc_t[64:128, :], sw_t[64:128, :])

                    # HAM prewarm: data-independent matmuls during the input
                    # DMAs so the PE clock is at full rate from the start.
                    warm = ps.tile([128, CHUNK], F32, tag="kq", bufs=2)
                    for w in range(NWARM):
                        nc.tensor.matmul(
                            warm[:, 0:128], ident[:], ident[:],
                            start=True, stop=True,
                        )

                    last_attn_inst = None
                    xc_next = xc0_g
                    for j in range(NCHUNK):
                        js = slice(j * CHUNK, (j + 1) * CHUNK)

                        # ================= A_j: projections + rope ==========
                        xc_g = xc_next
                        if j + 1 < NCHUNK:
                            # prefetch the next x chunk now, BEFORE B_j's
                            # staging triggers enter the DMA FIFOs (staging
                            # waits on attention results and would head-block
                            # these loads until B_j's end)
                            xc_next = load_xchunk(j + 1)

                        k_ps = ps.tile([128, CHUNK], F32, tag="kq", bufs=2,
                                       name=f"k_ps{j}")
                        for ko in range(NKO):
                            nc.tensor.matmul(
                                k_ps[:],
                                wk_g[ko // 8][:, ko % 8, :],
                                xc_g[ko // 8][:, ko % 8, :],
                                start=(ko == 0), stop=(ko == NKO - 1),
                            )
                        rope((ksb[0:64, js], ksb[64:128, js]), k_ps, js)

                        for h in range(HLOC):
                            q_ps = ps.tile([128, CHUNK], F32, tag="kq", bufs=2,
                                           name=f"q_ps{j}_{h}")
                            for ko in range(NKO):
                                nc.tensor.matmul(
                                    q_ps[:],
                                    wq_g[ko // 8][:, ko % 8, h * 128:(h + 1) * 128],
                                    xc_g[ko // 8][:, ko % 8, :],
                                    start=(ko == 0), stop=(ko == NKO - 1),
                                )
                            rope((qsb[0:64, h, js], qsb[64:128, h, js]), q_ps, js)

                        v_ps = ps.tile([128, CHUNK], F32, tag="kq", bufs=2,
                                       name=f"v_ps{j}")
                        for ko in range(NKO):
                            nc.tensor.matmul(
                                v_ps[:],
                                wv_g[ko // 8][:, ko % 8, :],
                                xc_g[ko // 8][:, ko % 8, :],
                                start=(ko == 0), stop=(ko == NKO - 1),
                            )
                        vT_sb = mp.tile([128, CHUNK], BF16, tag="vtsb",
                                        name=f"vt{j}")
                        nc.scalar.activation(
                            vT_sb[:], v_ps[:], mybir.ActivationFunctionType.Copy
                        )
                        # transpose via the DMA xbar (off the TensorEngine)
                        for t in range(4):
                            nc.scalar.dma_start_transpose(
                                vsb[:, 4 * j + t, :],
                                vT_sb[:, t * 128:(t + 1) * 128],
                            )

                        if j == 3:
                            # gathered-attn chunk 0: on the gpsimd queue,
                            # ordered after xc3's loads; it blocks on AG01
                            # completion but nothing critical sits behind it.
                            ag_tiles[0] = load_ag(
                                0, nc.gpsimd, last_xc_dma[(3, "gps")]
                            )

                        # ================= B_j: attention for chunk j =======
                        nblk = 4 * (j + 1)
                        for hp in range(2):
                            h0, h1 = 2 * hp, 2 * hp + 1
                            o = {}
                            for h in (h0, h1):
                                o[h] = ps.tile(
                                    [128, CHUNK], F32, tag="o", bufs=2,
                                    name=f"o_{j}_{h}",
                                )
                            dnacc = mp.tile(
                                [128, 2, CHUNK], BF16, tag="dnacc", bufs=1,
                                name=f"dna_{j}_{hp}",
                            )
                            pts = {}

                            def issue_sc(i, j=j, hp=hp, h0=h0, h1=h1, pts=pts):
                                # Diagonal blocks at offset t>=1 have columns
                                # < 128*t fully masked: trim the matmul/exp
                                # free dim to the valid range. Partial-width
                                # accumulating matmuls are safe: has_written
                                # is per-element and block i=0 (always full
                                # width) clears the bank with start=True.
                                t = i - 4 * j
                                lo = 128 * t if t >= 1 else 0
                                sc2 = ps.tile(
                                    [128, 2, CHUNK], F32, tag="sc", bufs=2,
                                    name=f"sc_{j}_{hp}_{i}",
                                )
                                nc.tensor.matmul(
                                    sc2[:, 0, lo:CHUNK],
                                    ksb[:, i * 128:(i + 1) * 128],
                                    qsb[:, h0, j * CHUNK + lo:(j + 1) * CHUNK],
                                    start=True, stop=True,
                                )
                                nc.tensor.matmul(
                                    sc2[:, 1, lo:CHUNK],
                                    ksb[:, i * 128:(i + 1) * 128],
                                    qsb[:, h1, j * CHUNK + lo:(j + 1) * CHUNK],
                                    start=True, stop=True,
                                )
                                pt2 = ptp.tile(
                                    [128, 2, CHUNK], BF16, tag="pt",
                                    name=f"pt_{j}_{hp}_{i}",
                                )
                                nc.scalar.activation(
                                    pt2[:, :, lo:CHUNK], sc2[:, :, lo:CHUNK],
                                    mybir.ActivationFunctionType.Exp,
                                    scale=SCALE,
                                )
                                if t >= 0:
                                    # causal mask: only the 128 columns of
                                    # the diagonal sub-block need zeroing
                                    nc.vector.tensor_mul(
                                        pt2[:, 0, lo:lo + 128],
                                        pt2[:, 0, lo:lo + 128],
                                        band[:, 384:512],
                                    )
                                    nc.vector.tensor_mul(
                                        pt2[:, 1, lo:lo + 128],
                                        pt2[:, 1, lo:lo + 128],
                                        band[:, 384:512],
                                    )
                                pts[i] = (pt2, lo)

                            SKEW = 2
                            for i in range(min(SKEW, nblk)):
                                issue_sc(i)
                            for i in range(nblk):
                                if i + SKEW < nblk:
                                    issue_sc(i + SKEW)
                                pt2, lo = pts.pop(i)
                                # both heads' PV share lhsT=vsb[:,i,:]
                                nc.tensor.matmul(
                                    o[h0][:, lo:CHUNK], vsb[:, i, :],
                                    pt2[:, 0, lo:CHUNK],
                                    start=(i == 0), stop=(i == nblk - 1),
                                )
                                nc.tensor.matmul(
                                    o[h1][:, lo:CHUNK], vsb[:, i, :],
                                    pt2[:, 1, lo:CHUNK],
                                    start=(i == 0), stop=(i == nblk - 1),
                                )
                                # denominator accumulation off the PE: bf16
                                # DVE chain (~0.5us/block, under the PV pace)
                                if i == 0:
                                    nc.vector.tensor_copy(dnacc[:], pt2[:])
                                else:
                                    nc.vector.tensor_add(
                                        dnacc[:, :, lo:CHUNK],
                                        dnacc[:, :, lo:CHUNK],
                                        pt2[:, :, lo:CHUNK],
                                    )
                            # partition-reduce denominators: one tiny
                            # ones-matmul per head, straight off dnacc
                            dnp = {}
                            for idx, h in enumerate((h0, h1)):
                                dnp[h] = ps.tile(
                                    [128, CHUNK], F32, tag="kq", bufs=2,
                                    name=f"dnp_{j}_{h}",
                                )
                                nc.tensor.matmul(
                                    dnp[h][0:1, :], ones_sb[:],
                                    dnacc[:, idx, :],
                                    start=True, stop=True,
                                )
                            # evacuate psum on ACT, then normalize + stage
                            for h in (h0, h1):
                                oun = mp.tile([128, CHUNK], F32, tag="oun",
                                              name=f"oun_{j}_{h}")
                                nc.scalar.activation(
                                    oun[:], o[h][:],
                                    mybir.ActivationFunctionType.Copy,
                                )
                                recip = mp.tile([1, CHUNK], F32, tag="recip",
                                                name=f"rc_{j}_{h}")
                                nc.vector.reciprocal_approx_fast(
                                    recip[:], dnp[h][0:1, :]
                                )
                                rb = mp.tile([128, CHUNK], F32, tag="rb",
                                             name=f"rb_{j}_{h}")
                                nc.gpsimd.partition_broadcast(rb[:], recip[:])
                                att = stp.tile([128, CHUNK], BF16, tag="att",
                                               name=f"att_{j}_{h}")
                                nc.vector.tensor_mul(att[:], oun[:], rb[:])
                                k, co = ag_slot(j)
                                last_attn_inst = nc.sync.dma_start(
                                    ag_in[k][h * 128:(h + 1) * 128,
                                             co:co + CHUNK],
                                    att[:],
                                )
                                staging_last[j] = last_attn_inst
                        if j >= 1:
                            k = ag_slot(j)[0]
                            nc.gpsimd.collective_compute(
                                "AllGather",
                                mybir.AluOpType.bypass,
                                replica_groups=rg,
                                ins=[ag_in[k][:].opt()],
                                outs=[ag_out[k][:].opt()],
                            )
                        if j == 3:
                            # gathered-attn chunks 1-3: sync queue, ordered
                            # after ALL staging so the blocked loads cannot
                            # delay the AG2/AG3 triggers.
                            ag_tiles[1] = load_ag(1, nc.sync, staging_last[3])
                            ag_tiles[2] = load_ag(2, nc.sync, staging_last[3])
                            ag_tiles[3] = load_ag(3, nc.sync, staging_last[3])

                # ---- phase C: output projection (column-parallel) ----
                with (
                    tc.tile_pool(name="wo", bufs=1) as wop,
                    tc.tile_pool(name="ost", bufs=3) as ostp,
                    tc.tile_pool(name="psC", bufs=2, space="PSUM") as psc,
                ):
                    wo_sb = wop.tile([128, NKO, ELOC], BF16)
                    for g in range(8):
                        ko = slice(4 * g, 4 * g + 4)
                        nc.scalar.dma_start(wo_sb[:, ko, :], woT[:, ko, :])
                    for j in range(NCHUNK):
                        js = slice(j * CHUNK, (j + 1) * CHUNK)
                        aggrp = ag_tiles[j]
                        for t in range(4):
                            wo_ps = psc.tile([128, CHUNK], F32, tag="wo")
                            for ko in range(NKO):
                                nc.tensor.matmul(
                                    wo_ps[:],
                                    wo_sb[:, ko, t * 128:(t + 1) * 128],
                                    aggrp[ko // 8][:, ko % 8, :],
                                    start=(ko == 0), stop=(ko == NKO - 1),
                                )
                            osb = ostp.tile([128, CHUNK], F32, tag="osb")
                            if j == 3 and t == 3:
                                # last tile: evacuate on the DVE and fan the
                                # write across both DMA pools for the tail
                                nc.vector.tensor_copy(osb[:], wo_ps[:])
                                engs = [nc.sync, nc.gpsimd, nc.sync, nc.gpsimd]
                                for q in range(4):
                                    c0 = 128 * q
                                    engs[q].dma_start(
                                        out_e.ap()[t * 128:(t + 1) * 128,
                                                   js.start + c0:js.start + c0 + 128],
                                        osb[:, c0:c0 + 128],
                                    )
                            else:
                                nc.scalar.activation(
                                    osb[:], wo_ps[:],
                                    mybir.ActivationFunctionType.Copy
                                )
                                nc.gpsimd.dma_start(
                                    out_e.ap()[t * 128:(t + 1) * 128,
                                               js.start:js.start + 256],
                                    osb[:, 0:256],
                                )
                                nc.gpsimd.dma_start(
                                    out_e.ap()[t * 128:(t + 1) * 128,
                                               js.start + 256:js.stop],
                                    osb[:, 256:CHUNK],
                                )

    nc.compile()
    return nc


def _get_nc():
    global _NC_CACHE
    if _NC_CACHE is None:
        _NC_CACHE = _build()
    return _NC_CACHE


_PERM = np.concatenate([np.arange(0, HD, 2), np.arange(1, HD, 2)])


def _pack_w(w_rows):
    """[m, D] fp32 row-major -> [128, NKO, m] bf16 partition-major."""
    wT = w_rows.T.astype(bf16)                     # [D, m]
    return np.ascontiguousarray(
        wT.reshape(NKO, 128, -1).transpose(1, 0, 2)
    )


def _prep_inputs(x, freqs_cos, freqs_sin, wq, wk, wv, wo):
    xT = np.ascontiguousarray(x.reshape(S, D).T.astype(bf16))
    cosT = np.ascontiguousarray(
        np.concatenate([freqs_cos.T, freqs_cos.T], axis=0).astype(np.float32)
    )
    sinT = np.ascontiguousarray(
        np.concatenate([freqs_sin.T, freqs_sin.T], axis=0).astype(np.float32)
    )
    in_maps = []
    for c in range(NCORES):
        heads = range(HLOC * c, HLOC * (c + 1))
        wq_c = np.concatenate(
            [wq[h * HD:(h + 1) * HD][_PERM] for h in heads], axis=0
        )  # [512, D] permuted
        wk_c = wk[c * HD:(c + 1) * HD][_PERM]
        wv_c = wv[c * HD:(c + 1) * HD]
        wo_c = wo[c * ELOC:(c + 1) * ELOC, :]      # [512, D] (rows = out dims)
        in_maps.append(
            {
                "xT": xT,
                "wqT": _pack_w(wq_c),
                "wkT": _pack_w(wk_c),
                "wvT": _pack_w(wv_c),
                "woT": _pack_w(wo_c),
                "cosT": cosT,
                "sinT": sinT,
            }
        )
    return in_maps


def _run(in_maps, trace=False, trace_cores=None):
    nc = _get_nc()
    return run_bass_kernel_spmd(
        nc,
        in_maps,
        list(range(NCORES)),
        trace=trace,
        trace_cores=trace_cores,
    )


def kernel(x, freqs_cos, freqs_sin, wq, wk, wv, wo):
    x = np.asarray(x, dtype=np.float32)
    in_maps = _prep_inputs(
        x,
        np.asarray(freqs_cos, np.float32),
        np.asarray(freqs_sin, np.float32),
        np.asarray(wq, np.float32),
        np.asarray(wk, np.float32),
        np.asarray(wv, np.float32),
        np.asarray(wo, np.float32),
    )
    res = _run(in_maps)
    out = np.empty((S, D), dtype=np.float32)
    for c in range(NCORES):
        out[:, c * ELOC:(c + 1) * ELOC] = np.asarray(
            res.results[c]["out"], dtype=np.float32
        ).T
    return out.reshape(B, S, D)


# revision 10
# speedup vs baseline: 1.0250x; 1.0250x over previous
"""Trainium2 8-core tensor-parallel Llama3-style GQA attention layer.

Problem: B=1, S=2048, D=4096, H=32 Q heads, KVH=8 KV heads, HD=128,
interleaved-pair RoPE (theta=5e5), causal softmax, output projection.

Sharding (Megatron TP-8):
  - core c owns Q heads [4c..4c+3] and KV head c (GQA groups align exactly),
  - x is replicated (passed pre-transposed as xT so the d-contraction sits on
    partitions with no on-device transposes),
  - wq/wk rows are permuted per head (even pair-indices first, then odd) so the
    interleaved RoPE becomes a "rotate-half" that is partition-aligned; the
    permutation cancels inside the q.k dot product,
  - weights are staged in DRAM partition-major ([128, ko, m]) so weight DMAs
    read 2-8KB contiguous lines instead of 256B gathers.

Schedule: projections (A_j) and attention (B_j) are interleaved per s-chunk
so each chunk's attention output stages ~130us earlier and the serialized
AllGather stream (~45-55us per op) fully hides under compute; the gathered
attention is consumed by a column-parallel wo projection (phase C) whose
input loads ride the same SBUF slots as the x chunks.

Attention details: scoresT[s2,s1] layout; the two heads of a GQA pair share
one [128, 2*CHUNK] score PSUM tile so exp is one ACT call per block; the
causal mask multiply only touches the 128 diagonal columns; softmax
denominators accumulate in bf16 on the DVE (off the TensorEngine) and are
partition-reduced by one tiny ones-matmul per (head, chunk); PSUM evacuation
runs on the scalar engine.  All of B's non-matmul work overlaps the next
chunk's projection matmuls.

kernel(**inputs) takes the FULL fp32 inputs and returns the FULL fp32 output.
"""

import sys

sys.path.insert(0, "/opt/trn_rl_repo")

import math

import numpy as np
import ml_dtypes

import concourse.bass as bass  # noqa: F401
import concourse.mybir as mybir
import concourse.tile as tile
from concourse import bacc
from concourse.bass_utils import run_bass_kernel_spmd
from concourse.masks import make_identity

bf16 = ml_dtypes.bfloat16
F32 = mybir.dt.float32
BF16 = mybir.dt.bfloat16

# Problem shapes (hardcoded per spec)
B, S, D = 1, 2048, 4096
H, KVH, HD = 32, 8, 128
NCORES = 8
HLOC = H // NCORES            # 4 q heads per core
ELOC = HLOC * HD              # 512 attn-out dims per core
NKO = D // 128                # 32 k-tiles of the d contraction
CHUNK = 512                   # s-chunk (matmul free dim / psum bank)
NCHUNK = S // CHUNK           # 4
NB = S // 128                 # 16 s2 blocks
SCALE = 1.0 / math.sqrt(HD)
NWARM = 44                    # HAM-prewarm matmuls issued during startup DMAs

_NC_CACHE = None


def _build():
    nc = bacc.Bacc(
        "TRN2",
        target_bir_lowering=False,
        debug=False,
        enable_asserts=True,
        num_devices=NCORES,
    )
    xT_e = nc.dram_tensor("xT", [D, S], BF16, kind="ExternalInput")
    # weights staged partition-major: [p, ko, m] so DMA lines are contiguous
    wq_e = nc.dram_tensor("wqT", [128, NKO, ELOC], BF16, kind="ExternalInput")
    wk_e = nc.dram_tensor("wkT", [128, NKO, HD], BF16, kind="ExternalInput")
    wv_e = nc.dram_tensor("wvT", [128, NKO, HD], BF16, kind="ExternalInput")
    wo_e = nc.dram_tensor("woT", [128, NKO, ELOC], BF16, kind="ExternalInput")
    cos_e = nc.dram_tensor("cosT", [HD, S], F32, kind="ExternalInput")
    sin_e = nc.dram_tensor("sinT", [HD, S], F32, kind="ExternalInput")
    out_e = nc.dram_tensor("out", [ELOC, S], F32, kind="ExternalOutput")

    xT = xT_e.ap().rearrange("(ko p) s -> p ko s", p=128)       # [128, 32, 2048]
    wqT = wq_e.ap()                                             # [128, 32, 512]
    wkT = wk_e.ap()                                             # [128, 32, 128]
    wvT = wv_e.ap()
    woT = wo_e.ap()                                             # [128, 32, 512]

    rg = [list(range(NCORES))]

    with tile.TileContext(nc) as tc:
        with (
            tc.tile_pool(name="dram", bufs=1, space="DRAM") as dram_pool,
            tc.tile_pool(name="persist", bufs=1) as pp,
        ):
            # AllGather buffers: one 2-chunk AG for s-chunks 0+1, then one
            # per chunk for 2 and 3; each fires as soon as its staging lands.
            AG_WIDTHS = [2 * CHUNK, CHUNK, CHUNK]
            ag_in = [
                dram_pool.tile([ELOC, w], BF16, name=f"ag_in{k}")
                for k, w in enumerate(AG_WIDTHS)
            ]
            ag_out = [
                dram_pool.tile(
                    [NCORES * ELOC, w], BF16, name=f"ag_out{k}",
                    addr_space="Shared",
                )
                for k, w in enumerate(AG_WIDTHS)
            ]

            def ag_slot(j):
                """(ag index, column offset) for s-chunk j."""
                return (0, j * CHUNK) if j < 2 else (j - 1, 0)

            # ---- small constants ----
            ident = pp.tile([128, 128], BF16)
            make_identity(nc, ident[:])
            band = pp.tile([128, 896], BF16)
            nc.gpsimd.memset(band[:], 1.0)
            # band[p, u] = 1 iff u >= p + 384
            nc.gpsimd.affine_select(
                out=band[:], in_=band[:],
                compare_op=mybir.AluOpType.is_ge, fill=0.0,
                base=-384, channel_multiplier=-1, pattern=[[1, 896]],
            )
            ones_sb = pp.tile([128, 1], BF16)
            nc.gpsimd.memset(ones_sb[:], 1.0)

            cos_sb = pp.tile([128, S], F32)
            sin_sb = pp.tile([128, S], F32)

            # ---- persistent activations ----
            qsb = pp.tile([128, HLOC, S], BF16)     # roped qT per head
            ksb = pp.tile([128, S], BF16)           # roped kT
            vsb = pp.tile([128, NB, HD], BF16)      # v[s2-tile, :, hd]

            ag_tiles = {}
            staging_last = {}

            # x-chunk pool outlives the A/B pools: the phase-C gathered-attn
            # loads ride the same tags/slots (so their SBUF region is never
            # reused under them by the later wo pool).
            with tc.tile_pool(name="xch", bufs=1) as xp:

                last_xc_dma = {}

                def load_xchunk(j, fine=False):
                    js = slice(j * CHUNK, (j + 1) * CHUNK)
                    xc_g = []
                    for g in range(4):
                        t = xp.tile(
                            [128, 8, CHUNK], BF16, tag=f"xc{g}", bufs=2,
                            name=f"xc{j}_{g}",
                        )
                        eng = nc.gpsimd if g % 2 else nc.sync
                        step = 1 if (fine and g == 0) else (2 if fine else 4)
                        for s0 in range(0, 8, step):
                            dd = eng.dma_start(
                                t[:, s0:s0 + step, :],
                                xT[:, 8 * g + s0:8 * g + s0 + step, js],
                            )
                            last_xc_dma[(j, "gps" if g % 2 else "sync")] = dd
                        xc_g.append(t)
                    return xc_g

                def load_ag(cj, eng, gate):
                    """Gathered-attn chunk cj -> SBUF, riding the xc tags.
                    `gate` orders these after critical work in eng's FIFO
                    (they block on AllGather completion at the queue head)."""
                    k, co = ag_slot(cj)
                    agt = ag_out[k][:].rearrange("(ko p) s -> p ko s", p=128)
                    tiles = []
                    for g in range(4):
                        t = xp.tile(
                            [128, 8, CHUNK], BF16, tag=f"xc{g}", bufs=2,
                            name=f"agsb{cj}_{g}",
                        )
                        d1 = eng.dma_start(
                            t[:, 0:4, :], agt[:, 8 * g:8 * g + 4, co:co + CHUNK]
                        )
                        d2 = eng.dma_start(
                            t[:, 4:8, :],
                            agt[:, 8 * g + 4:8 * g + 8, co:co + CHUNK],
                        )
                        if gate is not None:
                            for dd in (d1, d2):
                                tile.add_dep_helper(
                                    dd.ins, gate.ins, sync=False,
                                    reason="ag loads after critical queue work",
                                )
                        tiles.append(t)
                    return tiles

                with (
                    tc.tile_pool(name="wq", bufs=1) as wqp,
                    tc.tile_pool(name="wkv", bufs=1) as wkvp,
                    tc.tile_pool(name="rope", bufs=2) as rp,
                    tc.tile_pool(name="pt", bufs=6) as ptp,
                    tc.tile_pool(name="misc", bufs=2) as mp,
                    tc.tile_pool(name="stage", bufs=3) as stp,
                    tc.tile_pool(name="ps", bufs=1, space="PSUM") as ps,
                ):
                    def grp_load(pool, dram_t, m, name, nsplit=1,
                                 gate=None):
                        tiles = []
                        for g in range(4):
                            t = pool.tile([128, 8, m], BF16, name=f"{name}{g}")
                            step = 8 // nsplit
                            for s0 in range(0, 8, step):
                                dd = nc.sync.dma_start(
                                    t[:, s0:s0 + step, :],
                                    dram_t[:, 8 * g + s0:8 * g + s0 + step, :],
                                )
                                if gate is not None:
                                    tile.add_dep_helper(
                                        dd.ins, gate.ins, sync=False,
                                        reason="weights after x chunk 0",
                                    )
                            tiles.append(t)
                        return tiles

                    # DMA issue order = queue service order: wk first (first
                    # consumer), then x chunk 0, then wq/cos/sin/wv (gated
                    # after xc0 so the scheduler can't starve the k chain).
                    wk_g = grp_load(wkvp, wkT, HD, "wk", nsplit=1)
                    xc0_g = load_xchunk(0, fine=True)
                    xc0_gate = last_xc_dma[(0, "sync")]
                    wq_g = grp_load(wqp, wqT, ELOC, "wq", nsplit=2,
                                    gate=xc0_gate)
                    for g in range(4):
                        sl = slice(g * 512, (g + 1) * 512)
                        for src_ap, dst in ((cos_e, cos_sb), (sin_e, sin_sb)):
                            dd = nc.sync.dma_start(
                                dst[:, sl], src_ap.ap()[:, sl]
                            )
                            tile.add_dep_helper(
                                dd.ins, xc0_gate.ins, sync=False,
                                reason="cos/sin after x chunk 0",
                            )
                    wv_g = grp_load(wkvp, wvT, HD, "wv", nsplit=1,
                                    gate=xc0_gate)

                    def rope(dst01, src_ps, js):
                        """dst01: (ap_lo, ap_hi) bf16 targets [64, 512] each.
                        src_ps: [128, 512] psum with the permuted projection."""
                        tc_t = rp.tile([128, CHUNK], F32, tag="ropec")
                        ts_t = rp.tile([128, CHUNK], F32, tag="ropes")
                        sw_t = rp.tile([128, CHUNK], F32, tag="ropew")
                        nc.vector.tensor_mul(tc_t[:], src_ps[:], cos_sb[:, js])
                        nc.vector.tensor_mul(ts_t[:], src_ps[:], sin_sb[:, js])
                        nc.gpsimd.dma_start(sw_t[0:64, :], ts_t[64:128, :])
                        nc.gpsimd.dma_start(sw_t[64:128, :], ts_t[0:64, :])
                        nc.vector.tensor_sub(dst01[0], tc_t[0:64, :], sw_t[0:64, :])
                        nc.vector.tensor_add(dst01[1], tc_t[64:128, :], sw_t[64:128, :])

                    # HAM prewarm: data-independent matmuls during the input
                    # DMAs so the PE clock is at full rate from the start.
                    warm = ps.tile([128, CHUNK], F32, tag="kq", bufs=2)
                    for w in range(NWARM):
                        nc.tensor.matmul(
                            warm[:, 0:128], ident[:], ident[:],
                            start=True, stop=True,
                        )

                    last_attn_inst = None
                    for j in range(NCHUNK):
                        js = slice(j * CHUNK, (j + 1) * CHUNK)

                        # ================= A_j: projections + rope ==========
                        xc_g = xc0_g if j == 0 else load_xchunk(j)

                        k_ps = ps.tile([128, CHUNK], F32, tag="kq", bufs=2,
                                       name=f"k_ps{j}")
                        for ko in range(NKO):
                            nc.tensor.matmul(
                                k_ps[:],
                                wk_g[ko // 8][:, ko % 8, :],
                                xc_g[ko // 8][:, ko % 8, :],
                                start=(ko == 0), stop=(ko == NKO - 1),
                            )
                        rope((ksb[0:64, js], ksb[64:128, js]), k_ps, js)

                        for h in range(HLOC):
                            q_ps = ps.tile([128, CHUNK], F32, tag="kq", bufs=2,
                                           name=f"q_ps{j}_{h}")
                            for ko in range(NKO):
                                nc.tensor.matmul(
                                    q_ps[:],
                                    wq_g[ko // 8][:, ko % 8, h * 128:(h + 1) * 128],
                                    xc_g[ko // 8][:, ko % 8, :],
                                    start=(ko == 0), stop=(ko == NKO - 1),
                                )
                            rope((qsb[0:64, h, js], qsb[64:128, h, js]), q_ps, js)

                        v_ps = ps.tile([128, CHUNK], F32, tag="kq", bufs=2,
                                       name=f"v_ps{j}")
                        for ko in range(NKO):
                            nc.tensor.matmul(
                                v_ps[:],
                                wv_g[ko // 8][:, ko % 8, :],
                                xc_g[ko // 8][:, ko % 8, :],
                                start=(ko == 0), stop=(ko == NKO - 1),
                            )
                        vT_sb = mp.tile([128, CHUNK], BF16, tag="vtsb",
                                        name=f"vt{j}")
                        nc.scalar.activation(
                            vT_sb[:], v_ps[:], mybir.ActivationFunctionType.Copy
                        )
                        # transpose via the DMA xbar (off the TensorEngine)
                        for t in range(4):
                            nc.scalar.dma_start_transpose(
                                vsb[:, 4 * j + t, :],
                                vT_sb[:, t * 128:(t + 1) * 128],
                            )

                        if j == 3:
                            # gathered-attn chunk 0: AG01 completed long ago;
                            # gate after B2's staging so the scheduler keeps
                            # it out of the early-A3 sync stream.
                            ag_tiles[0] = load_ag(0, nc.sync, staging_last[2])

                        # ================= B_j: attention for chunk j =======
                        nblk = 4 * (j + 1)
                        for hp in range(2):
                            h0, h1 = 2 * hp, 2 * hp + 1
                            o = {}
                            for h in (h0, h1):
                                o[h] = ps.tile(
                                    [128, CHUNK], F32, tag="o", bufs=2,
                                    name=f"o_{j}_{h}",
                                )
                            dnacc = mp.tile(
                                [128, 2, CHUNK], BF16, tag="dnacc", bufs=1,
                                name=f"dna_{j}_{hp}",
                            )
                            pts = {}

                            def issue_sc(i, j=j, hp=hp, h0=h0, h1=h1, pts=pts):
                                # Diagonal blocks at offset t>=1 have columns
                                # < 128*t fully masked: trim the matmul/exp
                                # free dim to the valid range. Partial-width
                                # accumulating matmuls are safe: has_written
                                # is per-element and block i=0 (always full
                                # width) clears the bank with start=True.
                                t = i - 4 * j
                                lo = 128 * t if t >= 1 else 0
                                sc2 = ps.tile(
                                    [128, 2, CHUNK], F32, tag="sc", bufs=2,
                                    name=f"sc_{j}_{hp}_{i}",
                                )
                                nc.tensor.matmul(
                                    sc2[:, 0, lo:CHUNK],
                                    ksb[:, i * 128:(i + 1) * 128],
                                    qsb[:, h0, j * CHUNK + lo:(j + 1) * CHUNK],
                                    start=True, stop=True,
                                )
                                nc.tensor.matmul(
                                    sc2[:, 1, lo:CHUNK],
                                    ksb[:, i * 128:(i + 1) * 128],
                                    qsb[:, h1, j * CHUNK + lo:(j + 1) * CHUNK],
                                    start=True, stop=True,
                                )
                                pt2 = ptp.tile(
                                    [128, 2, CHUNK], BF16, tag="pt",
                                    name=f"pt_{j}_{hp}_{i}",
                                )
                                nc.scalar.activation(
                                    pt2[:, :, lo:CHUNK], sc2[:, :, lo:CHUNK],
                                    mybir.ActivationFunctionType.Exp,
                                    scale=SCALE,
                                )
                                if t >= 0:
                                    # causal mask: only the 128 columns of
                                    # the diagonal sub-block need zeroing
                                    nc.vector.tensor_mul(
                                        pt2[:, 0, lo:lo + 128],
                                        pt2[:, 0, lo:lo + 128],
                                        band[:, 384:512],
                                    )
                                    nc.vector.tensor_mul(
                                        pt2[:, 1, lo:lo + 128],
                                        pt2[:, 1, lo:lo + 128],
                                        band[:, 384:512],
                                    )
                                pts[i] = (pt2, lo)

                            SKEW = 2
                            for i in range(min(SKEW, nblk)):
                                issue_sc(i)
                            for i in range(nblk):
                                if i + SKEW < nblk:
                                    issue_sc(i + SKEW)
                                pt2, lo = pts.pop(i)
                                # both heads' PV share lhsT=vsb[:,i,:]
                                nc.tensor.matmul(
                                    o[h0][:, lo:CHUNK], vsb[:, i, :],
                                    pt2[:, 0, lo:CHUNK],
                                    start=(i == 0), stop=(i == nblk - 1),
                                )
                                nc.tensor.matmul(
                                    o[h1][:, lo:CHUNK], vsb[:, i, :],
                                    pt2[:, 1, lo:CHUNK],
                                    start=(i == 0), stop=(i == nblk - 1),
                                )
                                # denominator accumulation off the PE: bf16
                                # DVE chain (~0.5us/block, under the PV pace)
                                if i == 0:
                                    nc.vector.tensor_copy(dnacc[:], pt2[:])
                                else:
                                    nc.vector.tensor_add(
                                        dnacc[:, :, lo:CHUNK],
                                        dnacc[:, :, lo:CHUNK],
                                        pt2[:, :, lo:CHUNK],
                                    )
                            # partition-reduce denominators: one tiny
                            # ones-matmul per head, straight off dnacc
                            dnp = {}
                            for idx, h in enumerate((h0, h1)):
                                dnp[h] = ps.tile(
                                    [128, CHUNK], F32, tag="kq", bufs=2,
                                    name=f"dnp_{j}_{h}",
                                )
                                nc.tensor.matmul(
                                    dnp[h][0:1, :], ones_sb[:],
                                    dnacc[:, idx, :],
                                    start=True, stop=True,
                                )
                            # evacuate psum on ACT, then normalize + stage
                            for h in (h0, h1):
                                oun = mp.tile([128, CHUNK], F32, tag="oun",
                                              name=f"oun_{j}_{h}")
                                nc.scalar.activation(
                                    oun[:], o[h][:],
                                    mybir.ActivationFunctionType.Copy,
                                )
                                recip = mp.tile([1, CHUNK], F32, tag="recip",
                                                name=f"rc_{j}_{h}")
                                nc.vector.reciprocal_approx_fast(
                                    recip[:], dnp[h][0:1, :]
                                )
                                rb = mp.tile([128, CHUNK], F32, tag="rb",
                                             name=f"rb_{j}_{h}")
                                nc.gpsimd.partition_broadcast(rb[:], recip[:])
                                att = stp.tile([128, CHUNK], BF16, tag="att",
                                               name=f"att_{j}_{h}")
                                nc.vector.tensor_mul(att[:], oun[:], rb[:])
                                k, co = ag_slot(j)
                                last_attn_inst = nc.scalar.dma_start(
                                    ag_in[k][h * 128:(h + 1) * 128,
                                             co:co + CHUNK],
                                    att[:],
                                )
                                staging_last[j] = last_attn_inst
                        if j >= 1:
                            k = ag_slot(j)[0]
                            nc.gpsimd.collective_compute(
                                "AllGather",
                                mybir.AluOpType.bypass,
                                replica_groups=rg,
                                ins=[ag_in[k][:].opt()],
                                outs=[ag_out[k][:].opt()],
                            )
                        if j == 3:
                            # gathered-attn chunks 1-3: sync queue, ordered
                            # after ALL staging so the blocked loads cannot
                            # delay the AG2/AG3 triggers.
                            ag_tiles[1] = load_ag(1, nc.sync, staging_last[3])
                            ag_tiles[2] = load_ag(2, nc.sync, staging_last[3])
                            ag_tiles[3] = load_ag(3, nc.sync, staging_last[3])

                # ---- phase C: output projection (column-parallel) ----
                with (
                    tc.tile_pool(name="wo", bufs=1) as wop,
                    tc.tile_pool(name="ost", bufs=3) as ostp,
                    tc.tile_pool(name="psC", bufs=2, space="PSUM") as psc,
                ):
                    wo_sb = wop.tile([128, NKO, ELOC], BF16)
                    for g in range(8):
                        ko = slice(4 * g, 4 * g + 4)
                        nc.scalar.dma_start(wo_sb[:, ko, :], woT[:, ko, :])
                    for j in range(NCHUNK):
                        js = slice(j * CHUNK, (j + 1) * CHUNK)
                        aggrp = ag_tiles[j]
                        for t in range(4):
                            wo_ps = psc.tile([128, CHUNK], F32, tag="wo")
                            for ko in range(NKO):
                                nc.tensor.matmul(
                                    wo_ps[:],
                                    wo_sb[:, ko, t * 128:(t + 1) * 128],
                                    aggrp[ko // 8][:, ko % 8, :],
                                    start=(ko == 0), stop=(ko == NKO - 1),
                                )
                            osb = ostp.tile([128, CHUNK], F32, tag="osb")
                            if j == 3 and t == 3:
                                # last tile: evacuate on the DVE and fan the
                                # write across both DMA pools for the tail
                                nc.vector.tensor_copy(osb[:], wo_ps[:])
                                engs = [nc.sync, nc.gpsimd, nc.sync, nc.gpsimd]
                                for q in range(4):
                                    c0 = 128 * q
                                    engs[q].dma_start(
                                        out_e.ap()[t * 128:(t + 1) * 128,
                                                   js.start + c0:js.start + c0 + 128],
                                        osb[:, c0:c0 + 128],
                                    )
                            else:
                                nc.scalar.activation(
                                    osb[:], wo_ps[:],
                                    mybir.ActivationFunctionType.Copy
                                )
                                nc.gpsimd.dma_start(
                                    out_e.ap()[t * 128:(t + 1) * 128,
                                               js.start:js.start + 256],
                                    osb[:, 0:256],
                                )
                                nc.gpsimd.dma_start(
                                    out_e.ap()[t * 128:(t + 1) * 128,
                                               js.start + 256:js.stop],
                                    osb[:, 256:CHUNK],
                                )

    nc.compile()
    return nc


def _get_nc():
    global _NC_CACHE
    if _NC_CACHE is None:
        _NC_CACHE = _build()
    return _NC_CACHE


_PERM = np.concatenate([np.arange(0, HD, 2), np.arange(1, HD, 2)])


def _pack_w(w_rows):
    """[m, D] fp32 row-major -> [128, NKO, m] bf16 partition-major."""
    wT = w_rows.T.astype(bf16)                     # [D, m]
    return np.ascontiguousarray(
        wT.reshape(NKO, 128, -1).transpose(1, 0, 2)
    )


def _prep_inputs(x, freqs_cos, freqs_sin, wq, wk, wv, wo):
    xT = np.ascontiguousarray(x.reshape(S, D).T.astype(bf16))
    cosT = np.ascontiguousarray(
        np.concatenate([freqs_cos.T, freqs_cos.T], axis=0).astype(np.float32)
    )
    sinT = np.ascontiguousarray(
        np.concatenate([freqs_sin.T, freqs_sin.T], axis=0).astype(np.float32)
    )
    in_maps = []
    for c in range(NCORES):
        heads = range(HLOC * c, HLOC * (c + 1))
        wq_c = np.concatenate(
            [wq[h * HD:(h + 1) * HD][_PERM] for h in heads], axis=0
        )  # [512, D] permuted
        wk_c = wk[c * HD:(c + 1) * HD][_PERM]
        wv_c = wv[c * HD:(c + 1) * HD]
        wo_c = wo[c * ELOC:(c + 1) * ELOC, :]      # [512, D] (rows = out dims)
        in_maps.append(
            {
                "xT": xT,
                "wqT": _pack_w(wq_c),
                "wkT": _pack_w(wk_c),
                "wvT": _pack_w(wv_c),
                "woT": _pack_w(wo_c),
                "cosT": cosT,
                "sinT": sinT,
            }
        )
    return in_maps


def _run(in_maps, trace=False, trace_cores=None):
    nc = _get_nc()
    return run_bass_kernel_spmd(
        nc,
        in_maps,
        list(range(NCORES)),
        trace=trace,
        trace_cores=trace_cores,
    )


def kernel(x, freqs_cos, freqs_sin, wq, wk, wv, wo):
    x = np.asarray(x, dtype=np.float32)
    in_maps = _prep_inputs(
        x,
        np.asarray(freqs_cos, np.float32),
        np.asarray(freqs_sin, np.float32),
        np.asarray(wq, np.float32),
        np.asarray(wk, np.float32),
        np.asarray(wv, np.float32),
        np.asarray(wo, np.float32),
    )
    res = _run(in_maps)
    out = np.empty((S, D), dtype=np.float32)
    for c in range(NCORES):
        out[:, c * ELOC:(c + 1) * ELOC] = np.asarray(
            res.results[c]["out"], dtype=np.float32
        ).T
    return out.reshape(B, S, D)


# revision 11
# speedup vs baseline: 1.0752x; 1.0490x over previous
"""Trainium2 8-core tensor-parallel Llama3-style GQA attention layer.

Problem: B=1, S=2048, D=4096, H=32 Q heads, KVH=8 KV heads, HD=128,
interleaved-pair RoPE (theta=5e5), causal softmax, output projection.

Sharding (Megatron TP-8):
  - core c owns Q heads [4c..4c+3] and KV head c (GQA groups align exactly),
  - x is replicated (passed pre-transposed as xT so the d-contraction sits on
    partitions with no on-device transposes),
  - wq/wk rows are permuted per head (even pair-indices first, then odd) so the
    interleaved RoPE becomes a "rotate-half" that is partition-aligned; the
    permutation cancels inside the q.k dot product,
  - weights are staged in DRAM partition-major ([128, ko, m]) so weight DMAs
    read 2-8KB contiguous lines instead of 256B gathers.

Schedule: projections (A_j) and attention (B_j) are interleaved per s-chunk
so each chunk's attention output stages ~130us earlier and the serialized
AllGather stream (~45-55us per op) fully hides under compute; the gathered
attention is consumed by a column-parallel wo projection (phase C) whose
input loads ride the same SBUF slots as the x chunks.

Attention details: scoresT[s2,s1] layout; the two heads of a GQA pair share
one [128, 2*CHUNK] score PSUM tile so exp is one ACT call per block; the
causal mask multiply only touches the 128 diagonal columns; softmax
denominators accumulate in bf16 on the DVE (off the TensorEngine) and are
partition-reduced by one tiny ones-matmul per (head, chunk); PSUM evacuation
runs on the scalar engine.  All of B's non-matmul work overlaps the next
chunk's projection matmuls.

kernel(**inputs) takes the FULL fp32 inputs and returns the FULL fp32 output.
"""

import sys

sys.path.insert(0, "/opt/trn_rl_repo")

import math

import numpy as np
import ml_dtypes

import concourse.bass as bass  # noqa: F401
import concourse.mybir as mybir
import concourse.tile as tile
from concourse import bacc
from concourse.bass_utils import run_bass_kernel_spmd
from concourse.masks import make_identity

bf16 = ml_dtypes.bfloat16
F32 = mybir.dt.float32
BF16 = mybir.dt.bfloat16

# Problem shapes (hardcoded per spec)
B, S, D = 1, 2048, 4096
H, KVH, HD = 32, 8, 128
NCORES = 8
HLOC = H // NCORES            # 4 q heads per core
ELOC = HLOC * HD              # 512 attn-out dims per core
NKO = D // 128                # 32 k-tiles of the d contraction
CHUNK = 512                   # s-chunk (matmul free dim / psum bank)
NCHUNK = S // CHUNK           # 4
NB = S // 128                 # 16 s2 blocks
SCALE = 1.0 / math.sqrt(HD)
NWARM = 44                    # HAM-prewarm matmuls issued during startup DMAs

_NC_CACHE = None


def _build():
    nc = bacc.Bacc(
        "TRN2",
        target_bir_lowering=False,
        debug=False,
        enable_asserts=True,
        num_devices=NCORES,
    )
    xT_e = nc.dram_tensor("xT", [D, S], BF16, kind="ExternalInput")
    # weights staged partition-major: [p, ko, m] so DMA lines are contiguous
    wq_e = nc.dram_tensor("wqT", [128, NKO, ELOC], BF16, kind="ExternalInput")
    wk_e = nc.dram_tensor("wkT", [128, NKO, HD], BF16, kind="ExternalInput")
    wv_e = nc.dram_tensor("wvT", [128, NKO, HD], BF16, kind="ExternalInput")
    wo_e = nc.dram_tensor("woT", [128, NKO, ELOC], BF16, kind="ExternalInput")
    cos_e = nc.dram_tensor("cosT", [HD, S], F32, kind="ExternalInput")
    sin_e = nc.dram_tensor("sinT", [HD, S], F32, kind="ExternalInput")
    out_e = nc.dram_tensor("out", [ELOC, S], F32, kind="ExternalOutput")

    xT = xT_e.ap().rearrange("(ko p) s -> p ko s", p=128)       # [128, 32, 2048]
    wqT = wq_e.ap()                                             # [128, 32, 512]
    wkT = wk_e.ap()                                             # [128, 32, 128]
    wvT = wv_e.ap()
    woT = wo_e.ap()                                             # [128, 32, 512]

    rg = [list(range(NCORES))]

    with tile.TileContext(nc) as tc:
        with (
            tc.tile_pool(name="dram", bufs=1, space="DRAM") as dram_pool,
            tc.tile_pool(name="persist", bufs=1) as pp,
        ):
            # AllGather buffers: one 2-chunk AG for s-chunks 0+1, then one
            # per chunk for 2 and 3; each fires as soon as its staging lands.
            AG_WIDTHS = [2 * CHUNK, CHUNK, CHUNK]
            ag_in = [
                dram_pool.tile([ELOC, w], BF16, name=f"ag_in{k}")
                for k, w in enumerate(AG_WIDTHS)
            ]
            ag_out = [
                dram_pool.tile(
                    [NCORES * ELOC, w], BF16, name=f"ag_out{k}",
                    addr_space="Shared",
                )
                for k, w in enumerate(AG_WIDTHS)
            ]

            def ag_slot(j):
                """(ag index, column offset) for s-chunk j."""
                return (0, j * CHUNK) if j < 2 else (j - 1, 0)

            # ---- small constants ----
            ident = pp.tile([128, 128], BF16)
            make_identity(nc, ident[:])
            band = pp.tile([128, 896], BF16)
            nc.gpsimd.memset(band[:], 1.0)
            # band[p, u] = 1 iff u >= p + 384
            nc.gpsimd.affine_select(
                out=band[:], in_=band[:],
                compare_op=mybir.AluOpType.is_ge, fill=0.0,
                base=-384, channel_multiplier=-1, pattern=[[1, 896]],
            )
            ones_sb = pp.tile([128, 1], BF16)
            nc.gpsimd.memset(ones_sb[:], 1.0)

            cos_sb = pp.tile([128, S], F32)
            sin_sb = pp.tile([128, S], F32)

            # ---- persistent activations ----
            qsb = pp.tile([128, HLOC, S], BF16)     # roped qT per head
            ksb = pp.tile([128, S], BF16)           # roped kT
            vsb = pp.tile([128, NB, HD], BF16)      # v[s2-tile, :, hd]

            ag_tiles = {}
            staging_last = {}

            # x-chunk pool outlives the A/B pools: the phase-C gathered-attn
            # loads ride the same tags/slots (so their SBUF region is never
            # reused under them by the later wo pool).
            with tc.tile_pool(name="xch", bufs=1) as xp:

                last_xc_dma = {}

                def load_xchunk(j, fine=False):
                    js = slice(j * CHUNK, (j + 1) * CHUNK)
                    xc_g = []
                    for g in range(4):
                        t = xp.tile(
                            [128, 8, CHUNK], BF16, tag=f"xc{g}", bufs=2,
                            name=f"xc{j}_{g}",
                        )
                        eng = nc.gpsimd if g % 2 else nc.sync
                        step = 1 if (fine and g == 0) else (2 if fine else 4)
                        for s0 in range(0, 8, step):
                            dd = eng.dma_start(
                                t[:, s0:s0 + step, :],
                                xT[:, 8 * g + s0:8 * g + s0 + step, js],
                            )
                            last_xc_dma[(j, "gps" if g % 2 else "sync")] = dd
                        xc_g.append(t)
                    return xc_g

                def load_ag(cj, eng, gate):
                    """Gathered-attn chunk cj -> SBUF, riding the xc tags.
                    `gate` orders these after critical work in eng's FIFO
                    (they block on AllGather completion at the queue head)."""
                    k, co = ag_slot(cj)
                    agt = ag_out[k][:].rearrange("(ko p) s -> p ko s", p=128)
                    tiles = []
                    for g in range(4):
                        t = xp.tile(
                            [128, 8, CHUNK], BF16, tag=f"xc{g}", bufs=2,
                            name=f"agsb{cj}_{g}",
                        )
                        d1 = eng.dma_start(
                            t[:, 0:4, :], agt[:, 8 * g:8 * g + 4, co:co + CHUNK]
                        )
                        d2 = eng.dma_start(
                            t[:, 4:8, :],
                            agt[:, 8 * g + 4:8 * g + 8, co:co + CHUNK],
                        )
                        if gate is not None:
                            for dd in (d1, d2):
                                tile.add_dep_helper(
                                    dd.ins, gate.ins, sync=False,
                                    reason="ag loads after critical queue work",
                                )
                        tiles.append(t)
                    return tiles

                with (
                    tc.tile_pool(name="wq", bufs=1) as wqp,
                    tc.tile_pool(name="wkv", bufs=1) as wkvp,
                    tc.tile_pool(name="rope", bufs=2) as rp,
                    tc.tile_pool(name="pt", bufs=6) as ptp,
                    tc.tile_pool(name="misc", bufs=2) as mp,
                    tc.tile_pool(name="stage", bufs=3) as stp,
                    tc.tile_pool(name="ps", bufs=1, space="PSUM") as ps,
                ):
                    def grp_load(pool, dram_t, m, name, nsplit=1,
                                 gate=None):
                        tiles = []
                        for g in range(4):
                            t = pool.tile([128, 8, m], BF16, name=f"{name}{g}")
                            step = 8 // nsplit
                            for s0 in range(0, 8, step):
                                dd = nc.sync.dma_start(
                                    t[:, s0:s0 + step, :],
                                    dram_t[:, 8 * g + s0:8 * g + s0 + step, :],
                                )
                                if gate is not None:
                                    tile.add_dep_helper(
                                        dd.ins, gate.ins, sync=False,
                                        reason="weights after x chunk 0",
                                    )
                            tiles.append(t)
                        return tiles

                    # DMA issue order = queue service order: wk first (first
                    # consumer), then x chunk 0, then wq/cos/sin/wv (gated
                    # after xc0 so the scheduler can't starve the k chain).
                    wk_g = grp_load(wkvp, wkT, HD, "wk", nsplit=1)
                    xc0_g = load_xchunk(0, fine=True)
                    xc0_gate = last_xc_dma[(0, "sync")]
                    wq_g = grp_load(wqp, wqT, ELOC, "wq", nsplit=2,
                                    gate=xc0_gate)
                    for g in range(4):
                        sl = slice(g * 512, (g + 1) * 512)
                        for src_ap, dst in ((cos_e, cos_sb), (sin_e, sin_sb)):
                            dd = nc.sync.dma_start(
                                dst[:, sl], src_ap.ap()[:, sl]
                            )
                            tile.add_dep_helper(
                                dd.ins, xc0_gate.ins, sync=False,
                                reason="cos/sin after x chunk 0",
                            )
                    wv_g = grp_load(wkvp, wvT, HD, "wv", nsplit=1,
                                    gate=xc0_gate)

                    def rope(dst01, src_ps, js):
                        """dst01: (ap_lo, ap_hi) bf16 targets [64, 512] each.
                        src_ps: [128, 512] psum with the permuted projection."""
                        tc_t = rp.tile([128, CHUNK], F32, tag="ropec")
                        ts_t = rp.tile([128, CHUNK], F32, tag="ropes")
                        sw_t = rp.tile([128, CHUNK], F32, tag="ropew")
                        nc.vector.tensor_mul(tc_t[:], src_ps[:], cos_sb[:, js])
                        nc.vector.tensor_mul(ts_t[:], src_ps[:], sin_sb[:, js])
                        nc.gpsimd.dma_start(sw_t[0:64, :], ts_t[64:128, :])
                        nc.gpsimd.dma_start(sw_t[64:128, :], ts_t[0:64, :])
                        nc.vector.tensor_sub(dst01[0], tc_t[0:64, :], sw_t[0:64, :])
                        nc.vector.tensor_add(dst01[1], tc_t[64:128, :], sw_t[64:128, :])

                    # HAM prewarm: data-independent matmuls during the input
                    # DMAs so the PE clock is at full rate from the start.
                    warm = ps.tile([128, CHUNK], F32, tag="kq", bufs=2)
                    for w in range(NWARM):
                        nc.tensor.matmul(
                            warm[:, 0:128], ident[:], ident[:],
                            start=True, stop=True,
                        )

                    last_attn_inst = None
                    for j in range(NCHUNK):
                        js = slice(j * CHUNK, (j + 1) * CHUNK)

                        # ================= A_j: projections + rope ==========
                        xc_g = xc0_g if j == 0 else load_xchunk(j)

                        k_ps = ps.tile([128, CHUNK], F32, tag="kq", bufs=2,
                                       name=f"k_ps{j}")
                        for ko in range(NKO):
                            nc.tensor.matmul(
                                k_ps[:],
                                wk_g[ko // 8][:, ko % 8, :],
                                xc_g[ko // 8][:, ko % 8, :],
                                start=(ko == 0), stop=(ko == NKO - 1),
                            )
                        rope((ksb[0:64, js], ksb[64:128, js]), k_ps, js)

                        for h in range(HLOC):
                            q_ps = ps.tile([128, CHUNK], F32, tag="kq", bufs=2,
                                           name=f"q_ps{j}_{h}")
                            for ko in range(NKO):
                                nc.tensor.matmul(
                                    q_ps[:],
                                    wq_g[ko // 8][:, ko % 8, h * 128:(h + 1) * 128],
                                    xc_g[ko // 8][:, ko % 8, :],
                                    start=(ko == 0), stop=(ko == NKO - 1),
                                )
                            rope((qsb[0:64, h, js], qsb[64:128, h, js]), q_ps, js)

                        v_ps = ps.tile([128, CHUNK], F32, tag="kq", bufs=2,
                                       name=f"v_ps{j}")
                        for ko in range(NKO):
                            nc.tensor.matmul(
                                v_ps[:],
                                wv_g[ko // 8][:, ko % 8, :],
                                xc_g[ko // 8][:, ko % 8, :],
                                start=(ko == 0), stop=(ko == NKO - 1),
                            )
                        vT_sb = mp.tile([128, CHUNK], BF16, tag="vtsb",
                                        name=f"vt{j}")
                        nc.scalar.activation(
                            vT_sb[:], v_ps[:], mybir.ActivationFunctionType.Copy
                        )
                        # transpose on the PE (~0.3us each; DMA-xbar
                        # transposes serialize against the collectives'
                        # DMA ring and stalled B's PV matmuls ~25us)
                        for t in range(4):
                            trp = ps.tile([128, 128], BF16, tag="o", bufs=2,
                                          name=f"tr{j}_{t}")
                            nc.tensor.transpose(
                                trp[:], vT_sb[:, t * 128:(t + 1) * 128],
                                ident[:],
                            )
                            nc.vector.tensor_copy(vsb[:, 4 * j + t, :], trp[:])

                        if j == 3:
                            # gathered-attn chunk 0: AG01 completed long ago;
                            # gate after B2's staging so the scheduler keeps
                            # it out of the early-A3 sync stream.
                            ag_tiles[0] = load_ag(0, nc.sync, staging_last[2])

                        # ================= B_j: attention for chunk j =======
                        nblk = 4 * (j + 1)
                        for hp in range(2):
                            h0, h1 = 2 * hp, 2 * hp + 1
                            o = {}
                            for h in (h0, h1):
                                o[h] = ps.tile(
                                    [128, CHUNK], F32, tag="o", bufs=2,
                                    name=f"o_{j}_{h}",
                                )
                            dnacc = mp.tile(
                                [128, 2, CHUNK], BF16, tag="dnacc", bufs=1,
                                name=f"dna_{j}_{hp}",
                            )
                            pts = {}

                            def issue_sc(i, j=j, hp=hp, h0=h0, h1=h1, pts=pts):
                                # Diagonal blocks at offset t>=1 have columns
                                # < 128*t fully masked: trim the matmul/exp
                                # free dim to the valid range. Partial-width
                                # accumulating matmuls are safe: has_written
                                # is per-element and block i=0 (always full
                                # width) clears the bank with start=True.
                                t = i - 4 * j
                                lo = 128 * t if t >= 1 else 0
                                sc2 = ps.tile(
                                    [128, 2, CHUNK], F32, tag="sc", bufs=2,
                                    name=f"sc_{j}_{hp}_{i}",
                                )
                                nc.tensor.matmul(
                                    sc2[:, 0, lo:CHUNK],
                                    ksb[:, i * 128:(i + 1) * 128],
                                    qsb[:, h0, j * CHUNK + lo:(j + 1) * CHUNK],
                                    start=True, stop=True,
                                )
                                nc.tensor.matmul(
                                    sc2[:, 1, lo:CHUNK],
                                    ksb[:, i * 128:(i + 1) * 128],
                                    qsb[:, h1, j * CHUNK + lo:(j + 1) * CHUNK],
                                    start=True, stop=True,
                                )
                                pt2 = ptp.tile(
                                    [128, 2, CHUNK], BF16, tag="pt",
                                    name=f"pt_{j}_{hp}_{i}",
                                )
                                nc.scalar.activation(
                                    pt2[:, :, lo:CHUNK], sc2[:, :, lo:CHUNK],
                                    mybir.ActivationFunctionType.Exp,
                                    scale=SCALE,
                                )
                                if t >= 0:
                                    # causal mask: only the 128 columns of
                                    # the diagonal sub-block need zeroing
                                    nc.vector.tensor_mul(
                                        pt2[:, 0, lo:lo + 128],
                                        pt2[:, 0, lo:lo + 128],
                                        band[:, 384:512],
                                    )
                                    nc.vector.tensor_mul(
                                        pt2[:, 1, lo:lo + 128],
                                        pt2[:, 1, lo:lo + 128],
                                        band[:, 384:512],
                                    )
                                pts[i] = (pt2, lo)

                            SKEW = 2
                            for i in range(min(SKEW, nblk)):
                                issue_sc(i)
                            for i in range(nblk):
                                if i + SKEW < nblk:
                                    issue_sc(i + SKEW)
                                pt2, lo = pts.pop(i)
                                # both heads' PV share lhsT=vsb[:,i,:]
                                nc.tensor.matmul(
                                    o[h0][:, lo:CHUNK], vsb[:, i, :],
                                    pt2[:, 0, lo:CHUNK],
                                    start=(i == 0), stop=(i == nblk - 1),
                                )
                                nc.tensor.matmul(
                                    o[h1][:, lo:CHUNK], vsb[:, i, :],
                                    pt2[:, 1, lo:CHUNK],
                                    start=(i == 0), stop=(i == nblk - 1),
                                )
                                # denominator accumulation off the PE: bf16
                                # DVE chain (~0.5us/block, under the PV pace)
                                if i == 0:
                                    nc.vector.tensor_copy(dnacc[:], pt2[:])
                                else:
                                    nc.vector.tensor_add(
                                        dnacc[:, :, lo:CHUNK],
                                        dnacc[:, :, lo:CHUNK],
                                        pt2[:, :, lo:CHUNK],
                                    )
                            # partition-reduce denominators: one tiny
                            # ones-matmul per head, straight off dnacc
                            dnp = {}
                            for idx, h in enumerate((h0, h1)):
                                dnp[h] = ps.tile(
                                    [128, CHUNK], F32, tag="kq", bufs=2,
                                    name=f"dnp_{j}_{h}",
                                )
                                nc.tensor.matmul(
                                    dnp[h][0:1, :], ones_sb[:],
                                    dnacc[:, idx, :],
                                    start=True, stop=True,
                                )
                            # evacuate psum on ACT, then normalize + stage
                            for h in (h0, h1):
                                oun = mp.tile([128, CHUNK], F32, tag="oun",
                                              name=f"oun_{j}_{h}")
                                nc.scalar.activation(
                                    oun[:], o[h][:],
                                    mybir.ActivationFunctionType.Copy,
                                )
                                recip = mp.tile([1, CHUNK], F32, tag="recip",
                                                name=f"rc_{j}_{h}")
                                nc.vector.reciprocal_approx_fast(
                                    recip[:], dnp[h][0:1, :]
                                )
                                rb = mp.tile([128, CHUNK], F32, tag="rb",
                                             name=f"rb_{j}_{h}")
                                nc.gpsimd.partition_broadcast(rb[:], recip[:])
                                att = stp.tile([128, CHUNK], BF16, tag="att",
                                               name=f"att_{j}_{h}")
                                nc.vector.tensor_mul(att[:], oun[:], rb[:])
                                k, co = ag_slot(j)
                                last_attn_inst = nc.scalar.dma_start(
                                    ag_in[k][h * 128:(h + 1) * 128,
                                             co:co + CHUNK],
                                    att[:],
                                )
                                staging_last[j] = last_attn_inst
                        if j >= 1:
                            k = ag_slot(j)[0]
                            nc.gpsimd.collective_compute(
                                "AllGather",
                                mybir.AluOpType.bypass,
                                replica_groups=rg,
                                ins=[ag_in[k][:].opt()],
                                outs=[ag_out[k][:].opt()],
                            )
                        if j == 3:
                            # gathered-attn chunks 1-3: sync queue, ordered
                            # after ALL staging so the blocked loads cannot
                            # delay the AG2/AG3 triggers.
                            ag_tiles[1] = load_ag(1, nc.sync, staging_last[3])
                            ag_tiles[2] = load_ag(2, nc.sync, staging_last[3])
                            ag_tiles[3] = load_ag(3, nc.sync, staging_last[3])

                # ---- phase C: output projection (column-parallel) ----
                with (
                    tc.tile_pool(name="wo", bufs=1) as wop,
                    tc.tile_pool(name="ost", bufs=3) as ostp,
                    tc.tile_pool(name="psC", bufs=2, space="PSUM") as psc,
                ):
                    wo_sb = wop.tile([128, NKO, ELOC], BF16)
                    for g in range(8):
                        ko = slice(4 * g, 4 * g + 4)
                        nc.scalar.dma_start(wo_sb[:, ko, :], woT[:, ko, :])
                    for j in range(NCHUNK):
                        js = slice(j * CHUNK, (j + 1) * CHUNK)
                        aggrp = ag_tiles[j]
                        for t in range(4):
                            wo_ps = psc.tile([128, CHUNK], F32, tag="wo")
                            for ko in range(NKO):
                                nc.tensor.matmul(
                                    wo_ps[:],
                                    wo_sb[:, ko, t * 128:(t + 1) * 128],
                                    aggrp[ko // 8][:, ko % 8, :],
                                    start=(ko == 0), stop=(ko == NKO - 1),
                                )
                            osb = ostp.tile([128, CHUNK], F32, tag="osb")
                            if j == 3 and t == 3:
                                # last tile: evacuate on the DVE and fan the
                                # write across both DMA pools for the tail
                                nc.vector.tensor_copy(osb[:], wo_ps[:])
                                engs = [nc.sync, nc.gpsimd, nc.sync, nc.gpsimd]
                                for q in range(4):
                                    c0 = 128 * q
                                    engs[q].dma_start(
                                        out_e.ap()[t * 128:(t + 1) * 128,
                                                   js.start + c0:js.start + c0 + 128],
                                        osb[:, c0:c0 + 128],
                                    )
                            else:
                                nc.scalar.activation(
                                    osb[:], wo_ps[:],
                                    mybir.ActivationFunctionType.Copy
                                )
                                nc.gpsimd.dma_start(
                                    out_e.ap()[t * 128:(t + 1) * 128,
                                               js.start:js.start + 256],
                                    osb[:, 0:256],
                                )
                                nc.gpsimd.dma_start(
                                    out_e.ap()[t * 128:(t + 1) * 128,
                                               js.start + 256:js.stop],
                                    osb[:, 256:CHUNK],
                                )

    nc.compile()
    return nc


def _get_nc():
    global _NC_CACHE
    if _NC_CACHE is None:
        _NC_CACHE = _build()
    return _NC_CACHE


_PERM = np.concatenate([np.arange(0, HD, 2), np.arange(1, HD, 2)])


def _pack_w(w_rows):
    """[m, D] fp32 row-major -> [128, NKO, m] bf16 partition-major."""
    wT = w_rows.T.astype(bf16)                     # [D, m]
    return np.ascontiguousarray(
        wT.reshape(NKO, 128, -1).transpose(1, 0, 2)
    )


def _prep_inputs(x, freqs_cos, freqs_sin, wq, wk, wv, wo):
    xT = np.ascontiguousarray(x.reshape(S, D).T.astype(bf16))
    cosT = np.ascontiguousarray(
        np.concatenate([freqs_cos.T, freqs_cos.T], axis=0).astype(np.float32)
    )
    sinT = np.ascontiguousarray(
        np.concatenate([freqs_sin.T, freqs_sin.T], axis=0).astype(np.float32)
    )
    in_maps = []
    for c in range(NCORES):
        heads = range(HLOC * c, HLOC * (c + 1))
        wq_c = np.concatenate(
            [wq[h * HD:(h + 1) * HD][_PERM] for h in heads], axis=0
        )  # [512, D] permuted
        wk_c = wk[c * HD:(c + 1) * HD][_PERM]
        wv_c = wv[c * HD:(c + 1) * HD]
        wo_c = wo[c * ELOC:(c + 1) * ELOC, :]      # [512, D] (rows = out dims)
        in_maps.append(
            {
                "xT": xT,
                "wqT": _pack_w(wq_c),
                "wkT": _pack_w(wk_c),
                "wvT": _pack_w(wv_c),
                "woT": _pack_w(wo_c),
                "cosT": cosT,
                "sinT": sinT,
            }
        )
    return in_maps


def _run(in_maps, trace=False, trace_cores=None):
    nc = _get_nc()
    return run_bass_kernel_spmd(
        nc,
        in_maps,
        list(range(NCORES)),
        trace=trace,
        trace_cores=trace_cores,
    )


def kernel(x, freqs_cos, freqs_sin, wq, wk, wv, wo):
    x = np.asarray(x, dtype=np.float32)
    in_maps = _prep_inputs(
        x,
        np.asarray(freqs_cos, np.float32),
        np.asarray(freqs_sin, np.float32),
        np.asarray(wq, np.float32),
        np.asarray(wk, np.float32),
        np.asarray(wv, np.float32),
        np.asarray(wo, np.float32),
    )
    res = _run(in_maps)
    out = np.empty((S, D), dtype=np.float32)
    for c in range(NCORES):
        out[:, c * ELOC:(c + 1) * ELOC] = np.asarray(
            res.results[c]["out"], dtype=np.float32
        ).T
    return out.reshape(B, S, D)


# revision 13
# speedup vs baseline: 1.1025x; 1.0254x over previous
"""Trainium2 8-core tensor-parallel Llama3-style GQA attention layer.

Problem: B=1, S=2048, D=4096, H=32 Q heads, KVH=8 KV heads, HD=128,
interleaved-pair RoPE (theta=5e5), causal softmax, output projection.

Sharding (Megatron TP-8):
  - core c owns Q heads [4c..4c+3] and KV head c (GQA groups align exactly),
  - x is replicated (passed pre-transposed as xT so the d-contraction sits on
    partitions with no on-device transposes),
  - wq/wk rows are permuted per head (even pair-indices first, then odd) so the
    interleaved RoPE becomes a "rotate-half" that is partition-aligned; the
    permutation cancels inside the q.k dot product,
  - weights are staged in DRAM partition-major ([128, ko, m]) so weight DMAs
    read 2-8KB contiguous lines instead of 256B gathers.

Schedule: projections (A_j) and attention (B_j) are interleaved per s-chunk
so each chunk's attention output stages ~130us earlier and the serialized
AllGather stream (~45-55us per op) fully hides under compute; the gathered
attention is consumed by a column-parallel wo projection (phase C) whose
input loads ride the same SBUF slots as the x chunks.

Attention details: scoresT[s2,s1] layout; the two heads of a GQA pair share
one [128, 2*CHUNK] score PSUM tile so exp is one ACT call per block; the
causal mask multiply only touches the 128 diagonal columns; softmax
denominators accumulate in bf16 on the DVE (off the TensorEngine) and are
partition-reduced by one tiny ones-matmul per (head, chunk); PSUM evacuation
runs on the scalar engine.  All of B's non-matmul work overlaps the next
chunk's projection matmuls.

kernel(**inputs) takes the FULL fp32 inputs and returns the FULL fp32 output.
"""

import sys

sys.path.insert(0, "/opt/trn_rl_repo")

import math

import numpy as np
import ml_dtypes

import concourse.bass as bass  # noqa: F401
import concourse.mybir as mybir
import concourse.tile as tile
from concourse import bacc
from concourse.bass_utils import run_bass_kernel_spmd
from concourse.masks import make_identity

bf16 = ml_dtypes.bfloat16
F32 = mybir.dt.float32
BF16 = mybir.dt.bfloat16

# Problem shapes (hardcoded per spec)
B, S, D = 1, 2048, 4096
H, KVH, HD = 32, 8, 128
NCORES = 8
HLOC = H // NCORES            # 4 q heads per core
ELOC = HLOC * HD              # 512 attn-out dims per core
NKO = D // 128                # 32 k-tiles of the d contraction
CHUNK = 512                   # s-chunk (matmul free dim / psum bank)
NCHUNK = S // CHUNK           # 4
NB = S // 128                 # 16 s2 blocks
SCALE = 1.0 / math.sqrt(HD)
NWARM = 44                    # HAM-prewarm matmuls issued during startup DMAs

_NC_CACHE = None


def _build():
    nc = bacc.Bacc(
        "TRN2",
        target_bir_lowering=False,
        debug=False,
        enable_asserts=True,
        num_devices=NCORES,
    )
    xT_e = nc.dram_tensor("xT", [D, S], BF16, kind="ExternalInput")
    # weights staged partition-major: [p, ko, m] so DMA lines are contiguous
    wq_e = nc.dram_tensor("wqT", [128, NKO, ELOC], BF16, kind="ExternalInput")
    wk_e = nc.dram_tensor("wkT", [128, NKO, HD], BF16, kind="ExternalInput")
    wv_e = nc.dram_tensor("wvT", [128, NKO, HD], BF16, kind="ExternalInput")
    wo_e = nc.dram_tensor("woT", [128, NKO, ELOC], BF16, kind="ExternalInput")
    cos_e = nc.dram_tensor("cosT", [HD, S], F32, kind="ExternalInput")
    sin_e = nc.dram_tensor("sinT", [HD, S], F32, kind="ExternalInput")
    out_e = nc.dram_tensor("out", [ELOC, S], F32, kind="ExternalOutput")

    xT = xT_e.ap().rearrange("(ko p) s -> p ko s", p=128)       # [128, 32, 2048]
    wqT = wq_e.ap()                                             # [128, 32, 512]
    wkT = wk_e.ap()                                             # [128, 32, 128]
    wvT = wv_e.ap()
    woT = wo_e.ap()                                             # [128, 32, 512]

    rg = [list(range(NCORES))]

    with tile.TileContext(nc) as tc:
        with (
            tc.tile_pool(name="dram", bufs=1, space="DRAM") as dram_pool,
            tc.tile_pool(name="persist", bufs=1) as pp,
        ):
            # AllGather buffers: one 2-chunk AG for s-chunks 0+1, then one
            # per chunk for 2 and 3; each fires as soon as its staging lands.
            AG_WIDTHS = [2 * CHUNK, CHUNK, CHUNK]
            ag_in = [
                dram_pool.tile([ELOC, w], BF16, name=f"ag_in{k}")
                for k, w in enumerate(AG_WIDTHS)
            ]
            ag_out = [
                dram_pool.tile(
                    [NCORES * ELOC, w], BF16, name=f"ag_out{k}",
                    addr_space="Shared",
                )
                for k, w in enumerate(AG_WIDTHS)
            ]

            def ag_slot(j):
                """(ag index, column offset) for s-chunk j."""
                return (0, j * CHUNK) if j < 2 else (j - 1, 0)

            # ---- small constants ----
            ident = pp.tile([128, 128], BF16)
            make_identity(nc, ident[:])
            band = pp.tile([128, 896], BF16)
            nc.gpsimd.memset(band[:], 1.0)
            # band[p, u] = 1 iff u >= p + 384
            nc.gpsimd.affine_select(
                out=band[:], in_=band[:],
                compare_op=mybir.AluOpType.is_ge, fill=0.0,
                base=-384, channel_multiplier=-1, pattern=[[1, 896]],
            )
            ones_sb = pp.tile([128, 1], BF16)
            nc.gpsimd.memset(ones_sb[:], 1.0)

            cos_sb = pp.tile([128, S], F32)
            sin_sb = pp.tile([128, S], F32)

            # ---- persistent activations ----
            qsb = pp.tile([128, HLOC, S], BF16)     # roped qT per head
            ksb = pp.tile([128, S], BF16)           # roped kT
            vsb = pp.tile([128, NB, HD], BF16)      # v[s2-tile, :, hd]

            ag_tiles = {}
            wo_tiles = []
            staging_last = {}

            # x-chunk and wq pools outlive the A/B pools: phase C's
            # gathered-attn loads ride the xc tags and the wo weights ride
            # the wq tags, so their SBUF regions are never reused under them.
            with (
                tc.tile_pool(name="xch", bufs=1) as xp,
                tc.tile_pool(name="wq", bufs=1) as wqp,
            ):

                last_xc_dma = {}

                def load_xchunk(j, fine=False):
                    js = slice(j * CHUNK, (j + 1) * CHUNK)
                    xc_g = []
                    for g in range(4):
                        t = xp.tile(
                            [128, 8, CHUNK], BF16, tag=f"xc{g}", bufs=2,
                            name=f"xc{j}_{g}",
                        )
                        eng = nc.gpsimd if g % 2 else nc.sync
                        step = 1 if (fine and g == 0) else (2 if fine else 4)
                        for s0 in range(0, 8, step):
                            dd = eng.dma_start(
                                t[:, s0:s0 + step, :],
                                xT[:, 8 * g + s0:8 * g + s0 + step, js],
                            )
                            last_xc_dma[(j, "gps" if g % 2 else "sync")] = dd
                        xc_g.append(t)
                    return xc_g

                def load_ag(cj, eng, gate):
                    """Gathered-attn chunk cj -> SBUF, riding the xc tags.
                    `gate` orders these after critical work in eng's FIFO
                    (they block on AllGather completion at the queue head)."""
                    k, co = ag_slot(cj)
                    agt = ag_out[k][:].rearrange("(ko p) s -> p ko s", p=128)
                    tiles = []
                    for g in range(4):
                        t = xp.tile(
                            [128, 8, CHUNK], BF16, tag=f"xc{g}", bufs=2,
                            name=f"agsb{cj}_{g}",
                        )
                        d1 = eng.dma_start(
                            t[:, 0:4, :], agt[:, 8 * g:8 * g + 4, co:co + CHUNK]
                        )
                        d2 = eng.dma_start(
                            t[:, 4:8, :],
                            agt[:, 8 * g + 4:8 * g + 8, co:co + CHUNK],
                        )
                        if gate is not None:
                            for dd in (d1, d2):
                                tile.add_dep_helper(
                                    dd.ins, gate.ins, sync=False,
                                    reason="ag loads after critical queue work",
                                )
                        tiles.append(t)
                    return tiles

                with (
                    tc.tile_pool(name="wkv", bufs=1) as wkvp,
                    tc.tile_pool(name="rope", bufs=2) as rp,
                    tc.tile_pool(name="pt", bufs=6) as ptp,
                    tc.tile_pool(name="misc", bufs=2) as mp,
                    tc.tile_pool(name="stage", bufs=3) as stp,
                    tc.tile_pool(name="ps", bufs=1, space="PSUM") as ps,
                ):
                    def grp_load(pool, dram_t, m, name, nsplit=1,
                                 gate=None, tagbase=None):
                        tiles = []
                        for g in range(4):
                            t = pool.tile(
                                [128, 8, m], BF16, name=f"{name}{g}",
                                tag=(f"{tagbase}{g}" if tagbase else ""),
                            )
                            step = 8 // nsplit
                            for s0 in range(0, 8, step):
                                dd = nc.sync.dma_start(
                                    t[:, s0:s0 + step, :],
                                    dram_t[:, 8 * g + s0:8 * g + s0 + step, :],
                                )
                                if gate is not None:
                                    tile.add_dep_helper(
                                        dd.ins, gate.ins, sync=False,
                                        reason="weights after x chunk 0",
                                    )
                            tiles.append(t)
                        return tiles

                    # DMA issue order = queue service order: wk first (first
                    # consumer), then x chunk 0, then wq/cos/sin/wv (gated
                    # after xc0 so the scheduler can't starve the k chain).
                    wk_g = grp_load(wkvp, wkT, HD, "wk", nsplit=1)
                    xc0_g = load_xchunk(0, fine=True)
                    xc0_gate = last_xc_dma[(0, "sync")]
                    wq_g = grp_load(wqp, wqT, ELOC, "wq", nsplit=2,
                                    gate=xc0_gate, tagbase="wq")
                    for g in range(4):
                        sl = slice(g * 512, (g + 1) * 512)
                        for src_ap, dst in ((cos_e, cos_sb), (sin_e, sin_sb)):
                            dd = nc.sync.dma_start(
                                dst[:, sl], src_ap.ap()[:, sl]
                            )
                            tile.add_dep_helper(
                                dd.ins, xc0_gate.ins, sync=False,
                                reason="cos/sin after x chunk 0",
                            )
                    wv_g = grp_load(wkvp, wvT, HD, "wv", nsplit=1,
                                    gate=xc0_gate)

                    def rope(dst01, src_ps, js):
                        """dst01: (ap_lo, ap_hi) bf16 targets [64, 512] each.
                        src_ps: [128, 512] psum with the permuted projection."""
                        tc_t = rp.tile([128, CHUNK], F32, tag="ropec")
                        ts_t = rp.tile([128, CHUNK], F32, tag="ropes")
                        sw_t = rp.tile([128, CHUNK], F32, tag="ropew")
                        nc.vector.tensor_mul(tc_t[:], src_ps[:], cos_sb[:, js])
                        nc.vector.tensor_mul(ts_t[:], src_ps[:], sin_sb[:, js])
                        nc.gpsimd.dma_start(sw_t[0:64, :], ts_t[64:128, :])
                        nc.gpsimd.dma_start(sw_t[64:128, :], ts_t[0:64, :])
                        nc.vector.tensor_sub(dst01[0], tc_t[0:64, :], sw_t[0:64, :])
                        nc.vector.tensor_add(dst01[1], tc_t[64:128, :], sw_t[64:128, :])

                    # HAM prewarm: data-independent matmuls during the input
                    # DMAs so the PE clock is at full rate from the start.
                    warm = ps.tile([128, CHUNK], F32, tag="kq", bufs=2)
                    for w in range(NWARM):
                        nc.tensor.matmul(
                            warm[:, 0:128], ident[:], ident[:],
                            start=True, stop=True,
                        )

                    last_attn_inst = None
                    for j in range(NCHUNK):
                        js = slice(j * CHUNK, (j + 1) * CHUNK)

                        # ================= A_j: projections + rope ==========
                        xc_g = xc0_g if j == 0 else load_xchunk(j)

                        k_ps = ps.tile([128, CHUNK], F32, tag="kq", bufs=2,
                                       name=f"k_ps{j}")
                        for ko in range(NKO):
                            nc.tensor.matmul(
                                k_ps[:],
                                wk_g[ko // 8][:, ko % 8, :],
                                xc_g[ko // 8][:, ko % 8, :],
                                start=(ko == 0), stop=(ko == NKO - 1),
                            )
                        rope((ksb[0:64, js], ksb[64:128, js]), k_ps, js)

                        for h in range(HLOC):
                            q_ps = ps.tile([128, CHUNK], F32, tag="kq", bufs=2,
                                           name=f"q_ps{j}_{h}")
                            for ko in range(NKO):
                                nc.tensor.matmul(
                                    q_ps[:],
                                    wq_g[ko // 8][:, ko % 8, h * 128:(h + 1) * 128],
                                    xc_g[ko // 8][:, ko % 8, :],
                                    start=(ko == 0), stop=(ko == NKO - 1),
                                )
                            rope((qsb[0:64, h, js], qsb[64:128, h, js]), q_ps, js)

                        v_ps = ps.tile([128, CHUNK], F32, tag="kq", bufs=2,
                                       name=f"v_ps{j}")
                        for ko in range(NKO):
                            nc.tensor.matmul(
                                v_ps[:],
                                wv_g[ko // 8][:, ko % 8, :],
                                xc_g[ko // 8][:, ko % 8, :],
                                start=(ko == 0), stop=(ko == NKO - 1),
                            )
                        vT_sb = mp.tile([128, CHUNK], BF16, tag="vtsb",
                                        name=f"vt{j}")
                        nc.scalar.activation(
                            vT_sb[:], v_ps[:], mybir.ActivationFunctionType.Copy
                        )
                        # transpose on the PE (~0.3us each; DMA-xbar
                        # transposes serialize against the collectives'
                        # DMA ring and stalled B's PV matmuls ~25us)
                        for t in range(4):
                            trp = ps.tile([128, 128], BF16, tag="o", bufs=2,
                                          name=f"tr{j}_{t}")
                            nc.tensor.transpose(
                                trp[:], vT_sb[:, t * 128:(t + 1) * 128],
                                ident[:],
                            )
                            nc.vector.tensor_copy(vsb[:, 4 * j + t, :], trp[:])

                        if j == 3:
                            # gathered-attn chunk 0: AG01 completed long ago;
                            # gate after B2's staging so the scheduler keeps
                            # it out of the early-A3 sync stream.
                            ag_tiles[0] = load_ag(0, nc.sync, staging_last[2])
                            # wo weights ride the dying wq slots (sync queue;
                            # on the scalar queue these head-blocked B3's
                            # exps for ~39us waiting for the SBUF region)
                            wo_tiles.clear()
                            wo_tiles.extend(grp_load(
                                wqp, woT, ELOC, "wo", nsplit=2,
                                gate=staging_last[2], tagbase="wq",
                            ))

                        # ================= B_j: attention for chunk j =======
                        nblk = 4 * (j + 1)
                        for hp in range(2):
                            h0, h1 = 2 * hp, 2 * hp + 1
                            o = {}
                            for h in (h0, h1):
                                o[h] = ps.tile(
                                    [128, CHUNK], F32, tag="o", bufs=2,
                                    name=f"o_{j}_{h}",
                                )
                            dnacc = mp.tile(
                                [128, 2, CHUNK], BF16, tag="dnacc", bufs=1,
                                name=f"dna_{j}_{hp}",
                            )
                            pts = {}

                            def issue_sc(i, j=j, hp=hp, h0=h0, h1=h1, pts=pts):
                                # Diagonal blocks at offset t>=1 have columns
                                # < 128*t fully masked: trim the matmul/exp
                                # free dim to the valid range. Partial-width
                                # accumulating matmuls are safe: has_written
                                # is per-element and block i=0 (always full
                                # width) clears the bank with start=True.
                                t = i - 4 * j
                                lo = 128 * t if t >= 1 else 0
                                sc2 = ps.tile(
                                    [128, 2, CHUNK], F32, tag="sc", bufs=2,
                                    name=f"sc_{j}_{hp}_{i}",
                                )
                                nc.tensor.matmul(
                                    sc2[:, 0, lo:CHUNK],
                                    ksb[:, i * 128:(i + 1) * 128],
                                    qsb[:, h0, j * CHUNK + lo:(j + 1) * CHUNK],
                                    start=True, stop=True,
                                )
                                nc.tensor.matmul(
                                    sc2[:, 1, lo:CHUNK],
                                    ksb[:, i * 128:(i + 1) * 128],
                                    qsb[:, h1, j * CHUNK + lo:(j + 1) * CHUNK],
                                    start=True, stop=True,
                                )
                                pt2 = ptp.tile(
                                    [128, 2, CHUNK], BF16, tag="pt",
                                    name=f"pt_{j}_{hp}_{i}",
                                )
                                nc.scalar.activation(
                                    pt2[:, :, lo:CHUNK], sc2[:, :, lo:CHUNK],
                                    mybir.ActivationFunctionType.Exp,
                                    scale=SCALE,
                                )
                                if t >= 0:
                                    # causal mask: only the 128 columns of
                                    # the diagonal sub-block need zeroing
                                    nc.vector.tensor_mul(
                                        pt2[:, 0, lo:lo + 128],
                                        pt2[:, 0, lo:lo + 128],
                                        band[:, 384:512],
                                    )
                                    nc.vector.tensor_mul(
                                        pt2[:, 1, lo:lo + 128],
                                        pt2[:, 1, lo:lo + 128],
                                        band[:, 384:512],
                                    )
                                pts[i] = (pt2, lo)

                            SKEW = 2
                            for i in range(min(SKEW, nblk)):
                                issue_sc(i)
                            for i in range(nblk):
                                if i + SKEW < nblk:
                                    issue_sc(i + SKEW)
                                pt2, lo = pts.pop(i)
                                # both heads' PV share lhsT=vsb[:,i,:]
                                nc.tensor.matmul(
                                    o[h0][:, lo:CHUNK], vsb[:, i, :],
                                    pt2[:, 0, lo:CHUNK],
                                    start=(i == 0), stop=(i == nblk - 1),
                                )
                                nc.tensor.matmul(
                                    o[h1][:, lo:CHUNK], vsb[:, i, :],
                                    pt2[:, 1, lo:CHUNK],
                                    start=(i == 0), stop=(i == nblk - 1),
                                )
                                # denominator accumulation off the PE: bf16
                                # DVE chain (~0.5us/block, under the PV pace)
                                if i == 0:
                                    nc.vector.tensor_copy(dnacc[:], pt2[:])
                                else:
                                    nc.vector.tensor_add(
                                        dnacc[:, :, lo:CHUNK],
                                        dnacc[:, :, lo:CHUNK],
                                        pt2[:, :, lo:CHUNK],
                                    )
                            # partition-reduce denominators: one tiny
                            # ones-matmul per head, straight off dnacc
                            dnp = {}
                            for idx, h in enumerate((h0, h1)):
                                dnp[h] = ps.tile(
                                    [128, CHUNK], F32, tag="kq", bufs=2,
                                    name=f"dnp_{j}_{h}",
                                )
                                nc.tensor.matmul(
                                    dnp[h][0:1, :], ones_sb[:],
                                    dnacc[:, idx, :],
                                    start=True, stop=True,
                                )
                            # evacuate psum on ACT, then normalize + stage
                            for h in (h0, h1):
                                oun = mp.tile([128, CHUNK], F32, tag="oun",
                                              name=f"oun_{j}_{h}")
                                nc.scalar.activation(
                                    oun[:], o[h][:],
                                    mybir.ActivationFunctionType.Copy,
                                )
                                recip = mp.tile([1, CHUNK], F32, tag="recip",
                                                name=f"rc_{j}_{h}")
                                nc.vector.reciprocal_approx_fast(
                                    recip[:], dnp[h][0:1, :]
                                )
                                rb = mp.tile([128, CHUNK], F32, tag="rb",
                                             name=f"rb_{j}_{h}")
                                nc.gpsimd.partition_broadcast(rb[:], recip[:])
                                att = stp.tile([128, CHUNK], BF16, tag="att",
                                               name=f"att_{j}_{h}")
                                nc.vector.tensor_mul(att[:], oun[:], rb[:])
                                k, co = ag_slot(j)
                                last_attn_inst = nc.scalar.dma_start(
                                    ag_in[k][h * 128:(h + 1) * 128,
                                             co:co + CHUNK],
                                    att[:],
                                )
                                staging_last[j] = last_attn_inst
                        if j >= 1:
                            k = ag_slot(j)[0]
                            nc.gpsimd.collective_compute(
                                "AllGather",
                                mybir.AluOpType.bypass,
                                replica_groups=rg,
                                ins=[ag_in[k][:].opt()],
                                outs=[ag_out[k][:].opt()],
                            )
                        if j == 3:
                            # gathered-attn chunks 1-3: sync queue, ordered
                            # after ALL staging so the blocked loads cannot
                            # delay the AG2/AG3 triggers.
                            ag_tiles[1] = load_ag(1, nc.sync, staging_last[3])
                            ag_tiles[2] = load_ag(2, nc.sync, staging_last[3])
                            ag_tiles[3] = load_ag(3, nc.sync, staging_last[3])

                # ---- phase C: output projection (column-parallel) ----
                with (
                    tc.tile_pool(name="ost", bufs=3) as ostp,
                    tc.tile_pool(name="psC", bufs=2, space="PSUM") as psc,
                ):
                    for j in range(NCHUNK):
                        js = slice(j * CHUNK, (j + 1) * CHUNK)
                        aggrp = ag_tiles[j]
                        for t in range(4):
                            wo_ps = psc.tile([128, CHUNK], F32, tag="wo")
                            for ko in range(NKO):
                                nc.tensor.matmul(
                                    wo_ps[:],
                                    wo_tiles[ko // 8][:, ko % 8,
                                                      t * 128:(t + 1) * 128],
                                    aggrp[ko // 8][:, ko % 8, :],
                                    start=(ko == 0), stop=(ko == NKO - 1),
                                )
                            osb = ostp.tile([128, CHUNK], F32, tag="osb")
                            if j == 3 and t == 3:
                                # last tile: evacuate on the DVE and fan the
                                # write across both DMA pools for the tail
                                nc.vector.tensor_copy(osb[:], wo_ps[:])
                                engs = [nc.sync, nc.gpsimd, nc.sync, nc.gpsimd]
                                for q in range(4):
                                    c0 = 128 * q
                                    engs[q].dma_start(
                                        out_e.ap()[t * 128:(t + 1) * 128,
                                                   js.start + c0:js.start + c0 + 128],
                                        osb[:, c0:c0 + 128],
                                    )
                            else:
                                nc.scalar.activation(
                                    osb[:], wo_ps[:],
                                    mybir.ActivationFunctionType.Copy
                                )
                                nc.gpsimd.dma_start(
                                    out_e.ap()[t * 128:(t + 1) * 128,
                                               js.start:js.start + 256],
                                    osb[:, 0:256],
                                )
                                nc.gpsimd.dma_start(
                                    out_e.ap()[t * 128:(t + 1) * 128,
                                               js.start + 256:js.stop],
                                    osb[:, 256:CHUNK],
                                )

    nc.compile()
    return nc


def _get_nc():
    global _NC_CACHE
    if _NC_CACHE is None:
        _NC_CACHE = _build()
    return _NC_CACHE


_PERM = np.concatenate([np.arange(0, HD, 2), np.arange(1, HD, 2)])


def _pack_w(w_rows):
    """[m, D] fp32 row-major -> [128, NKO, m] bf16 partition-major."""
    wT = w_rows.T.astype(bf16)                     # [D, m]
    return np.ascontiguousarray(
        wT.reshape(NKO, 128, -1).transpose(1, 0, 2)
    )


def _prep_inputs(x, freqs_cos, freqs_sin, wq, wk, wv, wo):
    xT = np.ascontiguousarray(x.reshape(S, D).T.astype(bf16))
    cosT = np.ascontiguousarray(
        np.concatenate([freqs_cos.T, freqs_cos.T], axis=0).astype(np.float32)
    )
    sinT = np.ascontiguousarray(
        np.concatenate([freqs_sin.T, freqs_sin.T], axis=0).astype(np.float32)
    )
    in_maps = []
    for c in range(NCORES):
        heads = range(HLOC * c, HLOC * (c + 1))
        wq_c = np.concatenate(
            [wq[h * HD:(h + 1) * HD][_PERM] for h in heads], axis=0
        )  # [512, D] permuted
        wk_c = wk[c * HD:(c + 1) * HD][_PERM]
        wv_c = wv[c * HD:(c + 1) * HD]
        wo_c = wo[c * ELOC:(c + 1) * ELOC, :]      # [512, D] (rows = out dims)
        in_maps.append(
            {
                "xT": xT,
                "wqT": _pack_w(wq_c),
                "wkT": _pack_w(wk_c),
                "wvT": _pack_w(wv_c),
                "woT": _pack_w(wo_c),
                "cosT": cosT,
                "sinT": sinT,
            }
        )
    return in_maps


def _run(in_maps, trace=False, trace_cores=None):
    nc = _get_nc()
    return run_bass_kernel_spmd(
        nc,
        in_maps,
        list(range(NCORES)),
        trace=trace,
        trace_cores=trace_cores,
    )


def kernel(x, freqs_cos, freqs_sin, wq, wk, wv, wo):
    x = np.asarray(x, dtype=np.float32)
    in_maps = _prep_inputs(
        x,
        np.asarray(freqs_cos, np.float32),
        np.asarray(freqs_sin, np.float32),
        np.asarray(wq, np.float32),
        np.asarray(wk, np.float32),
        np.asarray(wv, np.float32),
        np.asarray(wo, np.float32),
    )
    res = _run(in_maps)
    out = np.empty((S, D), dtype=np.float32)
    for c in range(NCORES):
        out[:, c * ELOC:(c + 1) * ELOC] = np.asarray(
            res.results[c]["out"], dtype=np.float32
        ).T
    return out.reshape(B, S, D)


# revision 14
# speedup vs baseline: 1.2273x; 1.1132x over previous
"""Trainium2 8-core tensor-parallel Llama3-style GQA attention layer.

Problem: B=1, S=2048, D=4096, H=32 Q heads, KVH=8 KV heads, HD=128,
interleaved-pair RoPE (theta=5e5), causal softmax, output projection.

Sharding (Megatron TP-8):
  - core c owns Q heads [4c..4c+3] and KV head c (GQA groups align exactly),
  - x is replicated (passed pre-transposed as xT so the d-contraction sits on
    partitions with no on-device transposes),
  - wq/wk rows are permuted per head (even pair-indices first, then odd) so the
    interleaved RoPE becomes a "rotate-half" that is partition-aligned; the
    permutation cancels inside the q.k dot product,
  - weights are staged in DRAM partition-major ([128, ko, m]) so weight DMAs
    read 2-8KB contiguous lines instead of 256B gathers.

Schedule: projections (A_j) and attention (B_j) are interleaved per s-chunk
so each chunk's attention output stages ~130us earlier and the serialized
AllGather stream (~45-55us per op) fully hides under compute; the gathered
attention is consumed by a column-parallel wo projection (phase C) whose
input loads ride the same SBUF slots as the x chunks.

Attention details: scoresT[s2,s1] layout; the two heads of a GQA pair share
one [128, 2*CHUNK] score PSUM tile so exp is one ACT call per block; the
causal mask multiply only touches the 128 diagonal columns; softmax
denominators accumulate in bf16 on the DVE (off the TensorEngine) and are
partition-reduced by one tiny ones-matmul per (head, chunk); PSUM evacuation
runs on the scalar engine.  All of B's non-matmul work overlaps the next
chunk's projection matmuls.

kernel(**inputs) takes the FULL fp32 inputs and returns the FULL fp32 output.
"""

import sys

sys.path.insert(0, "/opt/trn_rl_repo")

import math

import numpy as np
import ml_dtypes

import concourse.bass as bass  # noqa: F401
import concourse.mybir as mybir
import concourse.tile as tile
from concourse import bacc
from concourse.bass_utils import run_bass_kernel_spmd
from concourse.masks import make_identity

bf16 = ml_dtypes.bfloat16
F32 = mybir.dt.float32
BF16 = mybir.dt.bfloat16

# Problem shapes (hardcoded per spec)
B, S, D = 1, 2048, 4096
H, KVH, HD = 32, 8, 128
NCORES = 8
HLOC = H // NCORES            # 4 q heads per core
ELOC = HLOC * HD              # 512 attn-out dims per core
NKO = D // 128                # 32 k-tiles of the d contraction
CHUNK = 512                   # s-chunk (matmul free dim / psum bank)
NCHUNK = S // CHUNK           # 4
NB = S // 128                 # 16 s2 blocks
SCALE = 1.0 / math.sqrt(HD)
NWARM = 72                    # HAM-prewarm matmuls issued during startup DMAs

_NC_CACHE = None


def _build():
    nc = bacc.Bacc(
        "TRN2",
        target_bir_lowering=False,
        debug=False,
        enable_asserts=True,
        num_devices=NCORES,
    )
    xT_e = nc.dram_tensor("xT", [D, S], BF16, kind="ExternalInput")
    # weights staged partition-major: [p, ko, m] so DMA lines are contiguous
    wq_e = nc.dram_tensor("wqT", [128, NKO, ELOC], BF16, kind="ExternalInput")
    wk_e = nc.dram_tensor("wkT", [128, NKO, HD], BF16, kind="ExternalInput")
    wv_e = nc.dram_tensor("wvT", [128, NKO, HD], BF16, kind="ExternalInput")
    wo_e = nc.dram_tensor("woT", [128, NKO, ELOC], BF16, kind="ExternalInput")
    cos_e = nc.dram_tensor("cosT", [HD, S], F32, kind="ExternalInput")
    sin_e = nc.dram_tensor("sinT", [HD, S], F32, kind="ExternalInput")
    out_e = nc.dram_tensor("out", [ELOC, S], F32, kind="ExternalOutput")

    xT = xT_e.ap().rearrange("(ko p) s -> p ko s", p=128)       # [128, 32, 2048]
    wqT = wq_e.ap()                                             # [128, 32, 512]
    wkT = wk_e.ap()                                             # [128, 32, 128]
    wvT = wv_e.ap()
    woT = wo_e.ap()                                             # [128, 32, 512]

    rg = [list(range(NCORES))]

    with tile.TileContext(nc) as tc:
        with (
            tc.tile_pool(name="dram", bufs=1, space="DRAM") as dram_pool,
            tc.tile_pool(name="persist", bufs=1) as pp,
        ):
            # AllGather buffers: one 2-chunk AG for s-chunks 0+1, then one
            # per chunk for 2 and 3; each fires as soon as its staging lands.
            AG_WIDTHS = [2 * CHUNK, CHUNK, CHUNK]
            ag_in = [
                dram_pool.tile([ELOC, w], BF16, name=f"ag_in{k}")
                for k, w in enumerate(AG_WIDTHS)
            ]
            ag_out = [
                dram_pool.tile(
                    [NCORES * ELOC, w], BF16, name=f"ag_out{k}",
                    addr_space="Shared",
                )
                for k, w in enumerate(AG_WIDTHS)
            ]

            def ag_slot(j):
                """(ag index, column offset) for s-chunk j."""
                return (0, j * CHUNK) if j < 2 else (j - 1, 0)

            # ---- small constants ----
            ident = pp.tile([128, 128], BF16)
            make_identity(nc, ident[:])
            band = pp.tile([128, 896], BF16)
            nc.gpsimd.memset(band[:], 1.0)
            # band[p, u] = 1 iff u >= p + 384
            nc.gpsimd.affine_select(
                out=band[:], in_=band[:],
                compare_op=mybir.AluOpType.is_ge, fill=0.0,
                base=-384, channel_multiplier=-1, pattern=[[1, 896]],
            )
            ones_sb = pp.tile([128, 1], BF16)
            nc.gpsimd.memset(ones_sb[:], 1.0)

            cos_sb = pp.tile([128, S], F32)
            sin_sb = pp.tile([128, S], F32)

            # ---- persistent activations ----
            qsb = pp.tile([128, HLOC, S], BF16)     # roped qT per head
            ksb = pp.tile([128, S], BF16)           # roped kT
            vsb = pp.tile([128, NB, HD], BF16)      # v[s2-tile, :, hd]

            ag_tiles = {}
            wo_tiles = []
            staging_last = {}

            # x-chunk and wq pools outlive the A/B pools: phase C's
            # gathered-attn loads ride the xc tags and the wo weights ride
            # the wq tags, so their SBUF regions are never reused under them.
            with (
                tc.tile_pool(name="xch", bufs=1) as xp,
                tc.tile_pool(name="wq", bufs=1) as wqp,
            ):

                last_xc_dma = {}

                def load_xchunk(j, fine=False):
                    js = slice(j * CHUNK, (j + 1) * CHUNK)
                    xc_g = []
                    for g in range(4):
                        t = xp.tile(
                            [128, 8, CHUNK], BF16, tag=f"xc{g}", bufs=2,
                            name=f"xc{j}_{g}",
                        )
                        eng = nc.gpsimd if g % 2 else nc.sync
                        step = 1 if (fine and g == 0) else (2 if fine else 4)
                        for s0 in range(0, 8, step):
                            dd = eng.dma_start(
                                t[:, s0:s0 + step, :],
                                xT[:, 8 * g + s0:8 * g + s0 + step, js],
                            )
                            last_xc_dma[(j, "gps" if g % 2 else "sync")] = dd
                        xc_g.append(t)
                    return xc_g

                def load_ag(cj, eng, gate):
                    """Gathered-attn chunk cj -> SBUF, riding the xc tags.
                    `gate` orders these after critical work in eng's FIFO
                    (they block on AllGather completion at the queue head)."""
                    k, co = ag_slot(cj)
                    agt = ag_out[k][:].rearrange("(ko p) s -> p ko s", p=128)
                    tiles = []
                    for g in range(4):
                        t = xp.tile(
                            [128, 8, CHUNK], BF16, tag=f"xc{g}", bufs=2,
                            name=f"agsb{cj}_{g}",
                        )
                        d1 = eng.dma_start(
                            t[:, 0:4, :], agt[:, 8 * g:8 * g + 4, co:co + CHUNK]
                        )
                        d2 = eng.dma_start(
                            t[:, 4:8, :],
                            agt[:, 8 * g + 4:8 * g + 8, co:co + CHUNK],
                        )
                        if gate is not None:
                            for dd in (d1, d2):
                                tile.add_dep_helper(
                                    dd.ins, gate.ins, sync=False,
                                    reason="ag loads after critical queue work",
                                )
                        tiles.append(t)
                    return tiles

                with (
                    tc.tile_pool(name="wkv", bufs=1) as wkvp,
                    tc.tile_pool(name="rope", bufs=2) as rp,
                    tc.tile_pool(name="pt", bufs=6) as ptp,
                    tc.tile_pool(name="misc", bufs=2) as mp,
                    tc.tile_pool(name="stage", bufs=3) as stp,
                    tc.tile_pool(name="ps", bufs=1, space="PSUM") as ps,
                ):
                    def grp_load(pool, dram_t, m, name, nsplit=1,
                                 gate=None, tagbase=None):
                        tiles = []
                        for g in range(4):
                            t = pool.tile(
                                [128, 8, m], BF16, name=f"{name}{g}",
                                tag=(f"{tagbase}{g}" if tagbase else ""),
                            )
                            step = 8 // nsplit
                            for s0 in range(0, 8, step):
                                dd = nc.sync.dma_start(
                                    t[:, s0:s0 + step, :],
                                    dram_t[:, 8 * g + s0:8 * g + s0 + step, :],
                                )
                                if gate is not None:
                                    tile.add_dep_helper(
                                        dd.ins, gate.ins, sync=False,
                                        reason="weights after x chunk 0",
                                    )
                            tiles.append(t)
                        return tiles

                    # DMA issue order = queue service order: wk first (first
                    # consumer), then x chunk 0, then wq/cos/sin/wv (gated
                    # after xc0 so the scheduler can't starve the k chain).
                    wk_g = grp_load(wkvp, wkT, HD, "wk", nsplit=1)
                    xc0_g = load_xchunk(0, fine=True)
                    xc0_gate = last_xc_dma[(0, "sync")]
                    wq_g = grp_load(wqp, wqT, ELOC, "wq", nsplit=2,
                                    gate=xc0_gate, tagbase="wq")
                    for g in range(4):
                        sl = slice(g * 512, (g + 1) * 512)
                        for src_ap, dst in ((cos_e, cos_sb), (sin_e, sin_sb)):
                            dd = nc.sync.dma_start(
                                dst[:, sl], src_ap.ap()[:, sl]
                            )
                            tile.add_dep_helper(
                                dd.ins, xc0_gate.ins, sync=False,
                                reason="cos/sin after x chunk 0",
                            )
                    wv_g = grp_load(wkvp, wvT, HD, "wv", nsplit=1,
                                    gate=xc0_gate)

                    def rope(dst01, src_ps, js):
                        """dst01: (ap_lo, ap_hi) bf16 targets [64, 512] each.
                        src_ps: [128, 512] psum with the permuted projection."""
                        tc_t = rp.tile([128, CHUNK], F32, tag="ropec")
                        ts_t = rp.tile([128, CHUNK], F32, tag="ropes")
                        sw_t = rp.tile([128, CHUNK], F32, tag="ropew")
                        nc.vector.tensor_mul(tc_t[:], src_ps[:], cos_sb[:, js])
                        nc.vector.tensor_mul(ts_t[:], src_ps[:], sin_sb[:, js])
                        nc.gpsimd.dma_start(sw_t[0:64, :], ts_t[64:128, :])
                        nc.gpsimd.dma_start(sw_t[64:128, :], ts_t[0:64, :])
                        nc.vector.tensor_sub(dst01[0], tc_t[0:64, :], sw_t[0:64, :])
                        nc.vector.tensor_add(dst01[1], tc_t[64:128, :], sw_t[64:128, :])

                    # HAM prewarm: data-independent matmuls during the input
                    # DMAs so the PE clock is at full rate from the start.
                    warm = ps.tile([128, CHUNK], F32, tag="kq", bufs=2)
                    for w in range(NWARM):
                        nc.tensor.matmul(
                            warm[:, 0:128], ident[:], ident[:],
                            start=True, stop=True,
                        )

                    last_attn_inst = None
                    for j in range(NCHUNK):
                        js = slice(j * CHUNK, (j + 1) * CHUNK)

                        # ================= A_j: projections + rope ==========
                        xc_g = xc0_g if j == 0 else load_xchunk(j)

                        k_ps = ps.tile([128, CHUNK], F32, tag="kq", bufs=2,
                                       name=f"k_ps{j}")
                        for ko in range(NKO):
                            nc.tensor.matmul(
                                k_ps[:],
                                wk_g[ko // 8][:, ko % 8, :],
                                xc_g[ko // 8][:, ko % 8, :],
                                start=(ko == 0), stop=(ko == NKO - 1),
                            )
                        rope((ksb[0:64, js], ksb[64:128, js]), k_ps, js)

                        for h in range(HLOC):
                            q_ps = ps.tile([128, CHUNK], F32, tag="kq", bufs=2,
                                           name=f"q_ps{j}_{h}")
                            for ko in range(NKO):
                                nc.tensor.matmul(
                                    q_ps[:],
                                    wq_g[ko // 8][:, ko % 8, h * 128:(h + 1) * 128],
                                    xc_g[ko // 8][:, ko % 8, :],
                                    start=(ko == 0), stop=(ko == NKO - 1),
                                )
                            rope((qsb[0:64, h, js], qsb[64:128, h, js]), q_ps, js)

                        v_ps = ps.tile([128, CHUNK], F32, tag="kq", bufs=2,
                                       name=f"v_ps{j}")
                        for ko in range(NKO):
                            nc.tensor.matmul(
                                v_ps[:],
                                wv_g[ko // 8][:, ko % 8, :],
                                xc_g[ko // 8][:, ko % 8, :],
                                start=(ko == 0), stop=(ko == NKO - 1),
                            )
                        vT_sb = mp.tile([128, CHUNK], BF16, tag="vtsb",
                                        name=f"vt{j}")
                        nc.scalar.activation(
                            vT_sb[:], v_ps[:], mybir.ActivationFunctionType.Copy
                        )
                        # transpose on the PE (~0.3us each; DMA-xbar
                        # transposes serialize against the collectives'
                        # DMA ring and stalled B's PV matmuls ~25us)
                        for t in range(4):
                            trp = ps.tile([128, 128], BF16, tag="o", bufs=2,
                                          name=f"tr{j}_{t}")
                            nc.tensor.transpose(
                                trp[:], vT_sb[:, t * 128:(t + 1) * 128],
                                ident[:],
                            )
                            nc.vector.tensor_copy(vsb[:, 4 * j + t, :], trp[:])

                        if j == 3:
                            # gathered-attn chunk 0: AG01 completed long ago;
                            # gate after B2's staging so the scheduler keeps
                            # it out of the early-A3 sync stream.
                            ag_tiles[0] = load_ag(0, nc.sync, staging_last[2])
                            # wo weights ride the dying wq slots (sync queue;
                            # on the scalar queue these head-blocked B3's
                            # exps for ~39us waiting for the SBUF region)
                            wo_tiles.clear()
                            wo_tiles.extend(grp_load(
                                wqp, woT, ELOC, "wo", nsplit=2,
                                gate=staging_last[2], tagbase="wq",
                            ))

                        # ================= B_j: attention for chunk j =======
                        nblk = 4 * (j + 1)
                        for hp in range(2):
                            h0, h1 = 2 * hp, 2 * hp + 1
                            o = {}
                            for h in (h0, h1):
                                o[h] = ps.tile(
                                    [128, CHUNK], F32, tag="o", bufs=2,
                                    name=f"o_{j}_{h}",
                                )
                            dnacc = mp.tile(
                                [128, 2, CHUNK], BF16, tag="dnacc", bufs=1,
                                name=f"dna_{j}_{hp}",
                            )
                            pts = {}

                            def issue_sc(i, j=j, hp=hp, h0=h0, h1=h1, pts=pts):
                                # Diagonal blocks at offset t>=1 have columns
                                # < 128*t fully masked: trim the matmul/exp
                                # free dim to the valid range. Partial-width
                                # accumulating matmuls are safe: has_written
                                # is per-element and block i=0 (always full
                                # width) clears the bank with start=True.
                                t = i - 4 * j
                                lo = 128 * t if t >= 1 else 0
                                sc2 = ps.tile(
                                    [128, 2, CHUNK], F32, tag="sc", bufs=2,
                                    name=f"sc_{j}_{hp}_{i}",
                                )
                                nc.tensor.matmul(
                                    sc2[:, 0, lo:CHUNK],
                                    ksb[:, i * 128:(i + 1) * 128],
                                    qsb[:, h0, j * CHUNK + lo:(j + 1) * CHUNK],
                                    start=True, stop=True,
                                )
                                nc.tensor.matmul(
                                    sc2[:, 1, lo:CHUNK],
                                    ksb[:, i * 128:(i + 1) * 128],
                                    qsb[:, h1, j * CHUNK + lo:(j + 1) * CHUNK],
                                    start=True, stop=True,
                                )
                                pt2 = ptp.tile(
                                    [128, 2, CHUNK], BF16, tag="pt",
                                    name=f"pt_{j}_{hp}_{i}",
                                )
                                nc.scalar.activation(
                                    pt2[:, :, lo:CHUNK], sc2[:, :, lo:CHUNK],
                                    mybir.ActivationFunctionType.Exp,
                                    scale=SCALE,
                                )
                                if t >= 0:
                                    # causal mask: only the 128 columns of
                                    # the diagonal sub-block need zeroing
                                    nc.vector.tensor_mul(
                                        pt2[:, 0, lo:lo + 128],
                                        pt2[:, 0, lo:lo + 128],
                                        band[:, 384:512],
                                    )
                                    nc.vector.tensor_mul(
                                        pt2[:, 1, lo:lo + 128],
                                        pt2[:, 1, lo:lo + 128],
                                        band[:, 384:512],
                                    )
                                pts[i] = (pt2, lo)

                            SKEW = 2
                            for i in range(min(SKEW, nblk)):
                                issue_sc(i)
                            for i in range(nblk):
                                if i + SKEW < nblk:
                                    issue_sc(i + SKEW)
                                pt2, lo = pts.pop(i)
                                # both heads' PV share lhsT=vsb[:,i,:]
                                nc.tensor.matmul(
                                    o[h0][:, lo:CHUNK], vsb[:, i, :],
                                    pt2[:, 0, lo:CHUNK],
                                    start=(i == 0), stop=(i == nblk - 1),
                                )
                                nc.tensor.matmul(
                                    o[h1][:, lo:CHUNK], vsb[:, i, :],
                                    pt2[:, 1, lo:CHUNK],
                                    start=(i == 0), stop=(i == nblk - 1),
                                )
                                # denominator accumulation off the PE: bf16
                                # DVE chain (~0.5us/block, under the PV pace)
                                if i == 0:
                                    nc.vector.tensor_copy(dnacc[:], pt2[:])
                                else:
                                    nc.vector.tensor_add(
                                        dnacc[:, :, lo:CHUNK],
                                        dnacc[:, :, lo:CHUNK],
                                        pt2[:, :, lo:CHUNK],
                                    )
                            # partition-reduce denominators: one tiny
                            # ones-matmul per head, straight off dnacc
                            dnp = {}
                            for idx, h in enumerate((h0, h1)):
                                dnp[h] = ps.tile(
                                    [128, CHUNK], F32, tag="o", bufs=2,
                                    name=f"dnp_{j}_{h}",
                                )
                                nc.tensor.matmul(
                                    dnp[h][0:1, :], ones_sb[:],
                                    dnacc[:, idx, :],
                                    start=True, stop=True,
                                )
                            # evacuate psum on ACT, then normalize + stage
                            for h in (h0, h1):
                                oun = mp.tile([128, CHUNK], F32, tag="oun",
                                              name=f"oun_{j}_{h}")
                                nc.scalar.activation(
                                    oun[:], o[h][:],
                                    mybir.ActivationFunctionType.Copy,
                                )
                                recip = mp.tile([1, CHUNK], F32, tag="recip",
                                                name=f"rc_{j}_{h}")
                                nc.vector.reciprocal_approx_fast(
                                    recip[:], dnp[h][0:1, :]
                                )
                                rb = mp.tile([128, CHUNK], F32, tag="rb",
                                             name=f"rb_{j}_{h}")
                                nc.gpsimd.partition_broadcast(rb[:], recip[:])
                                att = stp.tile([128, CHUNK], BF16, tag="att",
                                               name=f"att_{j}_{h}")
                                nc.vector.tensor_mul(att[:], oun[:], rb[:])
                                k, co = ag_slot(j)
                                last_attn_inst = nc.scalar.dma_start(
                                    ag_in[k][h * 128:(h + 1) * 128,
                                             co:co + CHUNK],
                                    att[:],
                                )
                                staging_last[j] = last_attn_inst
                        if j >= 1:
                            k = ag_slot(j)[0]
                            nc.gpsimd.collective_compute(
                                "AllGather",
                                mybir.AluOpType.bypass,
                                replica_groups=rg,
                                ins=[ag_in[k][:].opt()],
                                outs=[ag_out[k][:].opt()],
                            )
                        if j == 3:
                            # gathered-attn chunks 1-3: sync queue, ordered
                            # after ALL staging so the blocked loads cannot
                            # delay the AG2/AG3 triggers.
                            ag_tiles[1] = load_ag(1, nc.sync, staging_last[3])
                            ag_tiles[2] = load_ag(2, nc.sync, staging_last[3])
                            ag_tiles[3] = load_ag(3, nc.sync, staging_last[3])

                # ---- phase C: output projection (column-parallel) ----
                with (
                    tc.tile_pool(name="ost", bufs=3) as ostp,
                    tc.tile_pool(name="psC", bufs=2, space="PSUM") as psc,
                ):
                    for j in range(NCHUNK):
                        js = slice(j * CHUNK, (j + 1) * CHUNK)
                        aggrp = ag_tiles[j]
                        for t in range(4):
                            wo_ps = psc.tile([128, CHUNK], F32, tag="wo")
                            for ko in range(NKO):
                                nc.tensor.matmul(
                                    wo_ps[:],
                                    wo_tiles[ko // 8][:, ko % 8,
                                                      t * 128:(t + 1) * 128],
                                    aggrp[ko // 8][:, ko % 8, :],
                                    start=(ko == 0), stop=(ko == NKO - 1),
                                )
                            osb = ostp.tile([128, CHUNK], F32, tag="osb")
                            if j == 3 and t == 3:
                                # last tile: evacuate on the DVE and fan the
                                # write across both DMA pools for the tail
                                nc.vector.tensor_copy(osb[:], wo_ps[:])
                                engs = [nc.sync, nc.gpsimd, nc.sync, nc.gpsimd]
                                for q in range(4):
                                    c0 = 128 * q
                                    engs[q].dma_start(
                                        out_e.ap()[t * 128:(t + 1) * 128,
                                                   js.start + c0:js.start + c0 + 128],
                                        osb[:, c0:c0 + 128],
                                    )
                            else:
                                nc.scalar.activation(
                                    osb[:], wo_ps[:],
                                    mybir.ActivationFunctionType.Copy
                                )
                                nc.gpsimd.dma_start(
                                    out_e.ap()[t * 128:(t + 1) * 128,
                                               js.start:js.start + 256],
                                    osb[:, 0:256],
                                )
                                nc.gpsimd.dma_start(
                                    out_e.ap()[t * 128:(t + 1) * 128,
                                               js.start + 256:js.stop],
                                    osb[:, 256:CHUNK],
                                )

    nc.compile()
    return nc


def _get_nc():
    global _NC_CACHE
    if _NC_CACHE is None:
        _NC_CACHE = _build()
    return _NC_CACHE


_PERM = np.concatenate([np.arange(0, HD, 2), np.arange(1, HD, 2)])


def _pack_w(w_rows):
    """[m, D] fp32 row-major -> [128, NKO, m] bf16 partition-major."""
    wT = w_rows.T.astype(bf16)                     # [D, m]
    return np.ascontiguousarray(
        wT.reshape(NKO, 128, -1).transpose(1, 0, 2)
    )


def _prep_inputs(x, freqs_cos, freqs_sin, wq, wk, wv, wo):
    xT = np.ascontiguousarray(x.reshape(S, D).T.astype(bf16))
    cosT = np.ascontiguousarray(
        np.concatenate([freqs_cos.T, freqs_cos.T], axis=0).astype(np.float32)
    )
    sinT = np.ascontiguousarray(
        np.concatenate([freqs_sin.T, freqs_sin.T], axis=0).astype(np.float32)
    )
    in_maps = []
    for c in range(NCORES):
        heads = range(HLOC * c, HLOC * (c + 1))
        wq_c = np.concatenate(
            [wq[h * HD:(h + 1) * HD][_PERM] for h in heads], axis=0
        )  # [512, D] permuted
        wk_c = wk[c * HD:(c + 1) * HD][_PERM]
        wv_c = wv[c * HD:(c + 1) * HD]
        wo_c = wo[c * ELOC:(c + 1) * ELOC, :]      # [512, D] (rows = out dims)
        in_maps.append(
            {
                "xT": xT,
                "wqT": _pack_w(wq_c),
                "wkT": _pack_w(wk_c),
                "wvT": _pack_w(wv_c),
                "woT": _pack_w(wo_c),
                "cosT": cosT,
                "sinT": sinT,
            }
        )
    return in_maps


def _run(in_maps, trace=False, trace_cores=None):
    nc = _get_nc()
    return run_bass_kernel_spmd(
        nc,
        in_maps,
        list(range(NCORES)),
        trace=trace,
        trace_cores=trace_cores,
    )


def kernel(x, freqs_cos, freqs_sin, wq, wk, wv, wo):
    x = np.asarray(x, dtype=np.float32)
    in_maps = _prep_inputs(
        x,
        np.asarray(freqs_cos, np.float32),
        np.asarray(freqs_sin, np.float32),
        np.asarray(wq, np.float32),
        np.asarray(wk, np.float32),
        np.asarray(wv, np.float32),
        np.asarray(wo, np.float32),
    )
    res = _run(in_maps)
    out = np.empty((S, D), dtype=np.float32)
    for c in range(NCORES):
        out[:, c * ELOC:(c + 1) * ELOC] = np.asarray(
            res.results[c]["out"], dtype=np.float32
        ).T
    return out.reshape(B, S, D)
